# revision 26
# baseline (speedup 1.0000x reference)
"""GPT transformer block (B=4, T=1024, C=1024, H=16) on 8 Trainium2 cores.

Sharding: core = b*2 + qh  (b = batch element, qh = query-half of 512 tokens).
Each core computes K/V for its whole batch element (duplicated across the two
cores sharing it) and everything else — attention rows, attention-weight
output slice, MLP — for its own 512 query tokens.  No collectives; the host
concatenates the 8 output slices.

Matmuls run in bf16 with fp32 PSUM accumulation; layernorm, softmax and both
residual adds stay in fp32.
"""

from contextlib import ExitStack

import numpy as np
import ml_dtypes

import concourse.bacc as bacc
import concourse.mybir as mybir
from concourse.tile import TileContext
from concourse.bass_utils import run_bass_kernel_spmd

F32 = mybir.dt.float32
BF16 = mybir.dt.bfloat16
AF = mybir.ActivationFunctionType
ALU = mybir.AluOpType

B, T, C, H = 4, 1024, 1024, 16
HD = C // H          # 64
TQ = T // 2          # 512 query tokens per core
P = 128
NT = T // P          # 8 token tiles (full)
NQ = TQ // P         # 4 query token tiles
NCH = C // P         # 8 channel tiles
NF = 4 * C // P      # 32 ffn hidden tiles
EPS = 1e-5
SM_SCALE = 1.0 / np.sqrt(HD)

_BF = ml_dtypes.bfloat16


def _w4(w):
    """[M,K] weight -> [M/128, 128, K] bf16 lhsT-tile layout.

    out[m, p, kt*128+j] = w[m*128+j, kt*128+p]  (= w.T[kt*128+p, m*128+j])
    """
    M, K = w.shape
    nm, nk = M // P, K // P
    return np.ascontiguousarray(
        w.reshape(nm, P, nk, P).transpose(0, 3, 2, 1)).astype(_BF).reshape(
            nm, P, K)


def build_nc(use_mask: bool, use_ln_affine: bool, use_vb: bool):
    nc = bacc.Bacc(target_bir_lowering=False)

    d_x = nc.declare_dram_parameter("x_full", [T, C], F32, isOutput=False)
    d_xq = nc.declare_dram_parameter("xq", [TQ, C], F32, isOutput=False)
    d_wq = nc.declare_dram_parameter("wq4", [NCH, P, C], BF16, isOutput=False)
    d_wk = nc.declare_dram_parameter("wk4", [NCH, P, C], BF16, isOutput=False)
    d_wv = nc.declare_dram_parameter("wvr", [NCH, P, C], BF16, isOutput=False)
    d_wo = nc.declare_dram_parameter("wo4", [NCH, P, C], BF16, isOutput=False)
    d_wfc = nc.declare_dram_parameter("wfc4", [NF, P, C], BF16, isOutput=False)
    d_wpj = nc.declare_dram_parameter("wpj4", [NCH, P, 4 * C], BF16,
                                      isOutput=False)
    d_bqkv = nc.declare_dram_parameter("bqkv", [P, 3 * NCH], F32, isOutput=False)
    d_bo = nc.declare_dram_parameter("bo", [P, NCH], F32, isOutput=False)
    d_bfc = nc.declare_dram_parameter("bfc", [P, NF], F32, isOutput=False)
    d_bpj = nc.declare_dram_parameter("bpj", [P, NCH], F32, isOutput=False)
    d_idb = nc.declare_dram_parameter("ident_bf", [P, P], BF16, isOutput=False)
    d_idf = nc.declare_dram_parameter("ident_f", [P, P], F32, isOutput=False)
    if use_mask:
        d_mask = nc.declare_dram_parameter("mask_q", [TQ, T], F32,
                                           isOutput=False)
    if use_ln_affine:
        d_ln1 = nc.declare_dram_parameter("ln1_wb", [P, 2, C], F32,
                                          isOutput=False)
        d_ln2 = nc.declare_dram_parameter("ln2_wb", [P, 2, C], F32,
                                          isOutput=False)
    if use_vb:
        d_bvr = nc.declare_dram_parameter("bv_rep", [P, C], F32, isOutput=False)

    d_att = nc.declare_dram_parameter("att_out", [H, TQ, T], F32, isOutput=True)
    d_xo = nc.declare_dram_parameter("x_out", [TQ, C], F32, isOutput=True)

    with TileContext(nc) as tc, ExitStack() as top:
        small = top.enter_context(tc.tile_pool(name="small", bufs=1))
        big = top.enter_context(tc.tile_pool(name="big", bufs=1))
        ps_pool = top.enter_context(tc.tile_pool(name="ps", bufs=4,
                                                 space="PSUM"))
        sc_pool = top.enter_context(tc.tile_pool(name="sc", bufs=2,
                                                 space="PSUM"))

        consts = small.tile([P, 2], F32, tag="consts")
        id_bf = small.tile([P, P], BF16, tag="id_bf")
        id_f = small.tile([P, P], F32, tag="id_f")
        b_qkv = small.tile([P, 3 * NCH], F32, tag="b_qkv")
        b_o = small.tile([P, NCH], F32, tag="b_o")
        b_fc = small.tile([P, NF], F32, tag="b_fc")
        b_pj = small.tile([P, NCH], F32, tag="b_pj")

        # cross-phase tensors; disjoint lifetimes share a slot via the tag
        hT = big.tile([P, NT, T], BF16, tag="s_hT_gT")     # 16K  A..B
        gT = big.tile([P, NF, TQ], BF16, tag="s_hT_gT")    # 32K  E
        hqT = big.tile([P, NCH, TQ], BF16, tag="s_hqT_yT")  # 8K  A..B
        yT = big.tile([P, NCH, TQ], BF16, tag="s_hqT_yT")   # 8K  C..D
        kT = big.tile([P, NCH, T], BF16, tag="s_kT_xmid")  # 16K  B..C
        xmid = big.tile([P, NQ, C], F32, tag="s_kT_xmid")  # 16K  D..E
        qT = big.tile([P, NCH, TQ], BF16, tag="s_qT_h2T")   # 8K  B..C
        h2T = big.tile([P, NCH, TQ], BF16, tag="s_qT_h2T")  # 8K  D..E
        vtok = big.tile([P, NT, C], BF16, tag="s_vtok_pT")  # 16K B..C
        pT = big.tile([P, NCH, TQ], BF16, tag="s_vtok_pT")  # 8K  E

        nc.vector.memset(consts[:], 0.0)
        nc.vector.memset(consts[:, 0:1], EPS)
        eps_ap = consts[:, 0:1]
        nc.sync.dma_start(id_bf[:], d_idb[:])
        nc.sync.dma_start(id_f[:], d_idf[:])
        nc.sync.dma_start(b_qkv[:], d_bqkv[:])
        nc.sync.dma_start(b_o[:], d_bo[:])
        nc.sync.dma_start(b_fc[:], d_bfc[:])
        nc.sync.dma_start(b_pj[:], d_bpj[:])
        ln1_wb = ln2_wb = None
        if use_ln_affine:
            ln1_wb = small.tile([P, 2, C], F32, tag="ln1_wb")
            ln2_wb = small.tile([P, 2, C], F32, tag="ln2_wb")
            nc.sync.dma_start(ln1_wb[:], d_ln1[:])
            nc.sync.dma_start(ln2_wb[:], d_ln2[:])
        if use_vb:
            bv_rep = small.tile([P, C], F32, tag="bv_rep")
            nc.sync.dma_start(bv_rep[:], d_bvr[:])
        if use_mask:
            mask_sb = small.tile([P, NQ, T], F32, tag="mask")
            for tt in range(NQ):
                nc.sync.dma_start(mask_sb[:, tt, :],
                                  d_mask[tt * P:(tt + 1) * P, :])

        def ln_tile(pool, src_ap, dst_bf_ap, wb):
            """LayerNorm one [128, C] fp32 tile -> bf16 dst."""
            stats = pool.tile([P, 2, 6], F32, tag="ln_stats")
            mv = pool.tile([P, 2], F32, tag="ln_mv")
            for g in range(2):
                nc.vector.bn_stats(stats[:, g, :],
                                   src_ap[:, g * 512:(g + 1) * 512])
            nc.vector.bn_aggr(mv[:], stats[:])
            std = pool.tile([P, 1], F32, tag="ln_std")
            nc.scalar.activation(std[:], mv[:, 1:2], AF.Sqrt, bias=eps_ap)
            rs = pool.tile([P, 1], F32, tag="ln_rs")
            nc.vector.reciprocal(rs[:], std[:])
            if wb is None:
                # (x - mu) * rs == x*rs + (-mu*rs): do the wide pass on ACT
                nbias = pool.tile([P, 1], F32, tag="ln_nb")
                nc.vector.tensor_scalar(nbias[:], mv[:, 0:1], rs[:], -1.0,
                                        op0=ALU.mult, op1=ALU.mult)
                nc.scalar.activation(dst_bf_ap, src_ap, AF.Identity,
                                     bias=nbias[:], scale=rs[:])
            else:
                tmp = pool.tile([P, C], F32, tag="ln_tmp")
                nc.vector.tensor_scalar(tmp[:], src_ap, mv[:, 0:1], rs[:],
                                        op0=ALU.subtract, op1=ALU.mult)
                nc.vector.scalar_tensor_tensor(tmp[:], tmp[:], 1.0,
                                               wb[:, 0, :],
                                               op0=ALU.mult, op1=ALU.mult)
                nc.vector.tensor_add(dst_bf_ap, tmp[:], wb[:, 1, :])

        def transpose_bf(src_ap, dst_tile, dst_ci_base, dst_col0, n, eng):
            """PE-transpose n [128,128] bf16 blocks of src_ap (free offset
            i*128) into dst_tile[:, dst_ci_base+i, dst_col0:+128].
            Groups of 4 share one PSUM bank and one copy instruction."""
            for g0 in range(0, n, 4):
                gn = min(4, n - g0)
                pst = ps_pool.tile([P, 4 * P], BF16, tag="ps")
                for i in range(gn):
                    nc.tensor.transpose(
                        pst[:, i * P:(i + 1) * P],
                        src_ap[:, (g0 + i) * P:(g0 + i + 1) * P], id_bf[:])
                dst = dst_tile[:, dst_ci_base + g0:dst_ci_base + g0 + gn,
                               dst_col0:dst_col0 + P]
                src = pst[:, 0:gn * P].rearrange("p (g f) -> p g f", g=gn)
                if eng == "v":
                    nc.vector.tensor_copy(dst, src)
                else:
                    nc.scalar.copy(dst, src)

        # ================= Phase A: LN1 + transposes ====================
        with tc.tile_pool(name="phA", bufs=5) as pa:
            # xq first so the q projection can start while the rest of
            # phase A is still running
            for tt in range(NQ):
                xt = pa.tile([P, C], F32, tag="x_t")
                nc.sync.dma_start(xt[:], d_xq[tt * P:(tt + 1) * P, :])
                hqb = pa.tile([P, C], BF16, tag="h_b")
                ln_tile(pa, xt[:], hqb[:], ln1_wb)
                transpose_bf(hqb[:], hqT, 0, tt * P, 4, "v")
                transpose_bf(hqb[:, 4 * P:], hqT, 4, tt * P, 4, "s")
            for ti in range(NT):
                xt = pa.tile([P, C], F32, tag="x_t")
                nc.sync.dma_start(xt[:], d_x[ti * P:(ti + 1) * P, :])
                hb = pa.tile([P, C], BF16, tag="h_b")
                ln_tile(pa, xt[:], hb[:], ln1_wb)
                transpose_bf(hb[:], hT, 0, ti * P, 4, "v")
                transpose_bf(hb[:, 4 * P:], hT, 4, ti * P, 4, "s")

        # ================= Phase B: QKV projections =====================
        with tc.tile_pool(name="phB", bufs=5) as pb, \
             tc.tile_pool(name="phBv", bufs=1) as pbv:
            for m in range(NCH):
                wsb = pb.tile([P, C], BF16, tag="w_qk")
                nc.sync.dma_start(wsb[:], d_wq[m])
                ps = ps_pool.tile([P, TQ], F32, tag="ps")
                for kt in range(NCH):
                    nc.tensor.matmul(ps[:], wsb[:, kt * P:(kt + 1) * P],
                                     hqT[:, kt, :],
                                     start=(kt == 0), stop=(kt == NCH - 1))
                nc.scalar.activation(qT[:, m, :], ps[:], AF.Identity,
                                     bias=b_qkv[:, m:m + 1])
            for m in range(NCH):
                wsb = pb.tile([P, C], BF16, tag="w_qk")
                nc.sync.dma_start(wsb[:], d_wk[m])
                for nh in range(2):
                    ps = ps_pool.tile([P, 512], F32, tag="ps")
                    for kt in range(NCH):
                        nc.tensor.matmul(
                            ps[:], wsb[:, kt * P:(kt + 1) * P],
                            hT[:, kt, nh * 512:(nh + 1) * 512],
                            start=(kt == 0), stop=(kt == NCH - 1))
                    dst = kT[:, m, nh * 512:(nh + 1) * 512]
                    if (m + nh) % 2:
                        nc.scalar.activation(
                            dst, ps[:], AF.Identity,
                            bias=b_qkv[:, NCH + m:NCH + m + 1])
                    else:
                        nc.vector.tensor_scalar_add(
                            dst, ps[:], b_qkv[:, NCH + m:NCH + m + 1])
            wv_sb = pbv.tile([P, NCH, C], BF16, tag="w_v")
            for kt in range(NCH):
                nc.sync.dma_start(wv_sb[:, kt, :], d_wv[kt])
            for ti in range(NT):
                for nh in range(2):
                    ps = ps_pool.tile([P, 512], F32, tag="ps")
                    for kt in range(NCH):
                        nc.tensor.matmul(
                            ps[:], hT[:, kt, ti * P:(ti + 1) * P],
                            wv_sb[:, kt, nh * 512:(nh + 1) * 512],
                            start=(kt == 0), stop=(kt == NCH - 1))
                    dst = vtok[:, ti, nh * 512:(nh + 1) * 512]
                    if use_vb:
                        nc.vector.tensor_add(
                            dst, ps[:], bv_rep[:, nh * 512:(nh + 1) * 512])
                    elif (ti + nh) % 2:
                        nc.scalar.copy(dst, ps[:])
                    else:
                        nc.vector.tensor_copy(dst, ps[:])

        # ================= Phase C: attention ===========================
        # w_o weights prefetched here so phase D's matmuls are not stuck
        # behind the attention-output DMA backlog.
        es_wo = ExitStack()
        pdw = es_wo.enter_context(tc.tile_pool(name="phDw", bufs=NCH))
        wo_sb = []
        for m in range(NCH):
            w = pdw.tile([P, C], BF16, tag="w_o", name=f"w_o_{m}")
            nc.sync.dma_start(w[:], d_wo[m])
            wo_sb.append(w)

        # head pairs (2*hp, 2*hp+1) interleave so their K=64 score matmuls
        # land in PE row-groups 0 and 64 concurrently.
        with tc.tile_pool(name="phC", bufs=3) as pc_, \
             tc.tile_pool(name="phCt", bufs=2) as pct:
            for hp in range(H // 2):
                fi = hp
                attT2 = {0: pct.tile([P, NCH, TQ], BF16, tag="attT0",
                                     name=f"attT0_{hp}"),
                         HD: pct.tile([P, NCH, TQ], BF16, tag="attT1",
                                      name=f"attT1_{hp}")}
                for qt in range(NQ):
                    for po in (0, HD):
                        h = 2 * hp + (po // HD)
                        attT = attT2[po]
                        ssc = sc_pool.tile([P, T], F32, tag="sc")
                        for nh in range(2):
                            nc.tensor.matmul(
                                ssc[:, nh * 512:(nh + 1) * 512],
                                qT[po:po + HD, fi, qt * P:(qt + 1) * P],
                                kT[po:po + HD, fi, nh * 512:(nh + 1) * 512],
                                start=True, stop=True)
                        ex = pc_.tile([P, T], F32, tag="ex")
                        den = pc_.tile([P, 1], F32, tag="den")
                        if use_mask:
                            exin = pc_.tile([P, T], F32, tag="exin")
                            nc.vector.tensor_add(exin[:], ssc[:],
                                                 mask_sb[:, qt, :])
                            nc.scalar.activation(ex[:], exin[:], AF.Exp,
                                                 scale=SM_SCALE,
                                                 accum_out=den[:])
                        else:
                            nc.scalar.activation(ex[:], ssc[:], AF.Exp,
                                                 scale=SM_SCALE,
                                                 accum_out=den[:])
                        rec = pc_.tile([P, 1], F32, tag="rec")
                        nc.vector.reciprocal(rec[:], den[:])
                        att = pc_.tile([P, T], F32, tag="att")
                        nc.vector.tensor_scalar_mul(att[:], ex[:], rec[:])
                        nc.sync.dma_start(d_att[h, qt * P:(qt + 1) * P, :],
                                          att[:])
                        for g0 in range(0, NCH, 4):
                            pst = ps_pool.tile([P, 512], F32, tag="ps")
                            for i in range(4):
                                nc.tensor.transpose(
                                    pst[:, i * P:(i + 1) * P],
                                    att[:, (g0 + i) * P:(g0 + i + 1) * P],
                                    id_f[:])
                            dst = attT[:, g0:g0 + 4, qt * P:(qt + 1) * P]
                            src = pst[:].rearrange("p (g f) -> p g f", g=4)
                            if (qt + g0 // 4 + po // HD) % 2:
                                nc.scalar.copy(dst, src)
                            else:
                                nc.vector.tensor_copy(dst, src)
                for po in (0, HD):
                    ps_y = ps_pool.tile([HD, TQ], F32, tag="ps")
                    v0 = fi * P + po
                    for kt in range(NCH):
                        nc.tensor.matmul(ps_y[:],
                                         vtok[:, kt, v0:v0 + HD],
                                         attT2[po][:, kt, :],
                                         start=(kt == 0),
                                         stop=(kt == NCH - 1))
                    nc.vector.tensor_copy(yT[po:po + HD, fi, :], ps_y[:])

        # ============ Phase D: output proj + residual + LN2 =============
        with tc.tile_pool(name="phD", bufs=3) as pd_, \
             tc.tile_pool(name="phDao", bufs=1) as pdao:
            aoT = pdao.tile([P, NCH, TQ], BF16, tag="aoT")
            for m in range(NCH):
                wsb = wo_sb[m]
                ps = ps_pool.tile([P, TQ], F32, tag="ps")
                for kt in range(NCH):
                    nc.tensor.matmul(ps[:], wsb[:, kt * P:(kt + 1) * P],
                                     yT[:, kt, :],
                                     start=(kt == 0), stop=(kt == NCH - 1))
                nc.scalar.activation(aoT[:, m, :], ps[:], AF.Identity,
                                     bias=b_o[:, m:m + 1])
            for tt in range(NQ):
                xqt = pd_.tile([P, C], F32, tag="xq_t")
                nc.sync.dma_start(xqt[:], d_xq[tt * P:(tt + 1) * P, :])
                pst = ps_pool.tile([P, C], BF16, tag="ps")
                for m in range(NCH):
                    nc.tensor.transpose(pst[:, m * P:(m + 1) * P],
                                        aoT[:, m, tt * P:(tt + 1) * P],
                                        id_bf[:])
                nc.vector.tensor_add(xmid[:, tt, :], pst[:], xqt[:])
                h2b = pd_.tile([P, C], BF16, tag="h2_b")
                ln_tile(pd_, xmid[:, tt, :], h2b[:], ln2_wb)
                transpose_bf(h2b[:], h2T, 0, tt * P, 4, "v")
                transpose_bf(h2b[:, 4 * P:], h2T, 4, tt * P, 4, "s")
        es_wo.close()

        # ================= Phase E: FFN =================================
        with tc.tile_pool(name="phE", bufs=3) as pe, \
             tc.tile_pool(name="phEw", bufs=6) as pew:
            for m in range(NF):
                wsb = pew.tile([P, C], BF16, tag="w_fc")
                nc.sync.dma_start(wsb[:], d_wfc[m])
                ps = ps_pool.tile([P, TQ], F32, tag="ps")
                for kt in range(NCH):
                    nc.tensor.matmul(ps[:], wsb[:, kt * P:(kt + 1) * P],
                                     h2T[:, kt, :],
                                     start=(kt == 0), stop=(kt == NCH - 1))
                nc.scalar.activation(gT[:, m, :], ps[:], AF.Gelu,
                                     bias=b_fc[:, m:m + 1])
            for m in range(NCH):
                wsb = pe.tile([P, 4 * C], BF16, tag="w_pj")
                nc.sync.dma_start(wsb[:], d_wpj[m])
                ps = ps_pool.tile([P, TQ], F32, tag="ps")
                for kt in range(NF):
                    nc.tensor.matmul(ps[:], wsb[:, kt * P:(kt + 1) * P],
                                     gT[:, kt, :],
                                     start=(kt == 0), stop=(kt == NF - 1))
                nc.scalar.activation(pT[:, m, :], ps[:], AF.Identity,
                                     bias=b_pj[:, m:m + 1])
            for tt in range(NQ):
                pst = ps_pool.tile([P, C], BF16, tag="ps")
                for m in range(NCH):
                    nc.tensor.transpose(pst[:, m * P:(m + 1) * P],
                                        pT[:, m, tt * P:(tt + 1) * P],
                                        id_bf[:])
                xo = pe.tile([P, C], F32, tag="xo")
                nc.vector.tensor_add(xo[:], pst[:], xmid[:, tt, :])
                nc.sync.dma_start(d_xo[tt * P:(tt + 1) * P, :], xo[:])

    nc.compile()
    return nc


_NC_CACHE = {}
_LAST_IN_MAPS = None


def _get_nc(key):
    if key not in _NC_CACHE:
        _NC_CACHE[key] = build_nc(*key)
    return _NC_CACHE[key]


def kernel(x, attention_mask, ln1_w, ln1_b, w_qkv, b_qkv, w_o, b_o,
           ln2_w, ln2_b, w_fc, b_fc, w_proj, b_proj):
    x = np.asarray(x, np.float32)
    attention_mask = np.asarray(attention_mask, np.float32)
    ln1_w = np.asarray(ln1_w, np.float32)
    ln1_b = np.asarray(ln1_b, np.float32)
    w_qkv = np.asarray(w_qkv, np.float32)
    b_qkv_a = np.asarray(b_qkv, np.float32)
    w_o = np.asarray(w_o, np.float32)
    b_o_a = np.asarray(b_o, np.float32)
    ln2_w = np.asarray(ln2_w, np.float32)
    ln2_b = np.asarray(ln2_b, np.float32)
    w_fc = np.asarray(w_fc, np.float32)
    b_fc_a = np.asarray(b_fc, np.float32)
    w_proj = np.asarray(w_proj, np.float32)
    b_proj_a = np.asarray(b_proj, np.float32)

    use_mask = bool(np.any(attention_mask))
    use_ln_affine = not (np.all(ln1_w == 1) and np.all(ln1_b == 0)
                         and np.all(ln2_w == 1) and np.all(ln2_b == 0))
    use_vb = bool(np.any(b_qkv_a[2 * C:]))

    nc = _get_nc((use_mask, use_ln_affine, use_vb))

    shared = {
        "wq4": _w4(w_qkv[0:C]),
        "wk4": _w4(w_qkv[C:2 * C]),
        "wvr": np.ascontiguousarray(
            w_qkv[2 * C:3 * C].T.reshape(NCH, P, C)).astype(_BF),
        "wo4": _w4(w_o),
        "wfc4": _w4(w_fc),
        "wpj4": _w4(w_proj),
        "bqkv": np.ascontiguousarray(b_qkv_a.reshape(3 * NCH, P).T),
        "bo": np.ascontiguousarray(b_o_a.reshape(NCH, P).T),
        "bfc": np.ascontiguousarray(b_fc_a.reshape(NF, P).T),
        "bpj": np.ascontiguousarray(b_proj_a.reshape(NCH, P).T),
        "ident_bf": np.eye(P, dtype=_BF),
        "ident_f": np.eye(P, dtype=np.float32),
    }
    if use_ln_affine:
        shared["ln1_wb"] = np.ascontiguousarray(np.broadcast_to(
            np.stack([ln1_w, ln1_b]), (P, 2, C)))
        shared["ln2_wb"] = np.ascontiguousarray(np.broadcast_to(
            np.stack([ln2_w, ln2_b]), (P, 2, C)))
    if use_vb:
        shared["bv_rep"] = np.ascontiguousarray(
            np.broadcast_to(b_qkv_a[2 * C:], (P, C)))

    in_maps = []
    for core in range(8):
        b, qh = core // 2, core % 2
        m = dict(shared)
        m["x_full"] = np.ascontiguousarray(x[b])
        m["xq"] = np.ascontiguousarray(x[b, qh * TQ:(qh + 1) * TQ])
        if use_mask:
            # activation computes exp(scale*(s + m')), so pre-divide the
            # mask by scale to get exp(scale*s + mask).
            m["mask_q"] = np.ascontiguousarray(
                np.broadcast_to(attention_mask[0, 0], (T, T))
                [qh * TQ:(qh + 1) * TQ] / SM_SCALE)
        in_maps.append(m)

    global _LAST_IN_MAPS
    _LAST_IN_MAPS = in_maps
    res = run_bass_kernel_spmd(nc, in_maps, list(range(8)))

    x_out = np.empty((B, T, C), np.float32)
    att = np.empty((B, H, T, T), np.float32)
    for core in range(8):
        b, qh = core // 2, core % 2
        x_out[b, qh * TQ:(qh + 1) * TQ] = res.results[core]["x_out"]
        att[b, :, qh * TQ:(qh + 1) * TQ, :] = res.results[core]["att_out"]
    return (x_out, att)


# revision 27
# speedup vs baseline: 1.0429x; 1.0429x over previous
"""GPT transformer block (B=4, T=1024, C=1024, H=16) on 8 Trainium2 cores.

Sharding: core = b*2 + qh  (b = batch element, qh = query-half of 512 tokens).
Each core computes K/V for its whole batch element (duplicated across the two
cores sharing it) and everything else — attention rows, attention-weight
output slice, MLP — for its own 512 query tokens.  No collectives; the host
concatenates the 8 output slices.

Matmuls run in bf16 with fp32 PSUM accumulation; layernorm, softmax and both
residual adds stay in fp32.
"""

from contextlib import ExitStack

import numpy as np
import ml_dtypes

import concourse.bacc as bacc
import concourse.mybir as mybir
from concourse.tile import TileContext
from concourse.bass_utils import run_bass_kernel_spmd

F32 = mybir.dt.float32
BF16 = mybir.dt.bfloat16
AF = mybir.ActivationFunctionType
ALU = mybir.AluOpType

B, T, C, H = 4, 1024, 1024, 16
HD = C // H          # 64
TQ = T // 2          # 512 query tokens per core
P = 128
NT = T // P          # 8 token tiles (full)
NQ = TQ // P         # 4 query token tiles
NCH = C // P         # 8 channel tiles
NF = 4 * C // P      # 32 ffn hidden tiles
EPS = 1e-5
SM_SCALE = 1.0 / np.sqrt(HD)

_BF = ml_dtypes.bfloat16


def _w4(w):
    """[M,K] weight -> [M/128, 128, K] bf16 lhsT-tile layout.

    out[m, p, kt*128+j] = w[m*128+j, kt*128+p]  (= w.T[kt*128+p, m*128+j])
    """
    M, K = w.shape
    nm, nk = M // P, K // P
    return np.ascontiguousarray(
        w.reshape(nm, P, nk, P).transpose(0, 3, 2, 1)).astype(_BF).reshape(
            nm, P, K)


def build_nc(use_mask: bool, use_ln_affine: bool, use_vb: bool):
    nc = bacc.Bacc(target_bir_lowering=False)

    d_x = nc.declare_dram_parameter("x_full", [T, C], F32, isOutput=False)
    d_xq = nc.declare_dram_parameter("xq", [TQ, C], F32, isOutput=False)
    d_wq = nc.declare_dram_parameter("wq4", [NCH, P, C], BF16, isOutput=False)
    d_wk = nc.declare_dram_parameter("wk4", [NCH, P, C], BF16, isOutput=False)
    d_wv = nc.declare_dram_parameter("wvr", [NCH, P, C], BF16, isOutput=False)
    d_wo = nc.declare_dram_parameter("wo4", [NCH, P, C], BF16, isOutput=False)
    d_wfc = nc.declare_dram_parameter("wfc4", [NF, P, C], BF16, isOutput=False)
    d_wpj = nc.declare_dram_parameter("wpj4", [NCH, P, 4 * C], BF16,
                                      isOutput=False)
    d_bqkv = nc.declare_dram_parameter("bqkv", [P, 3 * NCH], F32, isOutput=False)
    d_bo = nc.declare_dram_parameter("bo", [P, NCH], F32, isOutput=False)
    d_bfc = nc.declare_dram_parameter("bfc", [P, NF], F32, isOutput=False)
    d_bpj = nc.declare_dram_parameter("bpj", [P, NCH], F32, isOutput=False)
    d_idb = nc.declare_dram_parameter("ident_bf", [P, P], BF16, isOutput=False)
    d_idf = nc.declare_dram_parameter("ident_f", [P, P], F32, isOutput=False)
    if use_mask:
        d_mask = nc.declare_dram_parameter("mask_q", [TQ, T], F32,
                                           isOutput=False)
    if use_ln_affine:
        d_ln1 = nc.declare_dram_parameter("ln1_wb", [P, 2, C], F32,
                                          isOutput=False)
        d_ln2 = nc.declare_dram_parameter("ln2_wb", [P, 2, C], F32,
                                          isOutput=False)
    if use_vb:
        d_bvr = nc.declare_dram_parameter("bv_rep", [P, C], F32, isOutput=False)

    d_att = nc.declare_dram_parameter("att_out", [H, TQ, T], F32, isOutput=True)
    d_xo = nc.declare_dram_parameter("x_out", [TQ, C], F32, isOutput=True)

    with TileContext(nc) as tc, ExitStack() as top:
        small = top.enter_context(tc.tile_pool(name="small", bufs=1))
        big = top.enter_context(tc.tile_pool(name="big", bufs=1))
        ps_pool = top.enter_context(tc.tile_pool(name="ps", bufs=4,
                                                 space="PSUM"))
        sc_pool = top.enter_context(tc.tile_pool(name="sc", bufs=2,
                                                 space="PSUM"))

        consts = small.tile([P, 2], F32, tag="consts")
        id_bf = small.tile([P, P], BF16, tag="id_bf")
        id_f = small.tile([P, P], F32, tag="id_f")
        b_qkv = small.tile([P, 3 * NCH], F32, tag="b_qkv")
        b_o = small.tile([P, NCH], F32, tag="b_o")
        b_fc = small.tile([P, NF], F32, tag="b_fc")
        b_pj = small.tile([P, NCH], F32, tag="b_pj")

        # cross-phase tensors; disjoint lifetimes share a slot via the tag
        hT = big.tile([P, NT, T], BF16, tag="s_hT_gT")     # 16K  A..B
        gT = big.tile([P, NF, TQ], BF16, tag="s_hT_gT")    # 32K  E
        hqT = big.tile([P, NCH, TQ], BF16, tag="s_hqT_yT")  # 8K  A..B
        yT = big.tile([P, NCH, TQ], BF16, tag="s_hqT_yT")   # 8K  C..D
        kT = big.tile([P, NCH, T], BF16, tag="s_kT_xmid")  # 16K  B..C
        xmid = big.tile([P, NQ, C], F32, tag="s_kT_xmid")  # 16K  D..E
        qT = big.tile([P, NCH, TQ], BF16, tag="s_qT_h2T")   # 8K  B..C
        h2T = big.tile([P, NCH, TQ], BF16, tag="s_qT_h2T")  # 8K  D..E
        vtok = big.tile([P, NT, C], BF16, tag="s_vtok_pT")  # 16K B..C
        pT = big.tile([P, NCH, TQ], BF16, tag="s_vtok_pT")  # 8K  E

        nc.vector.memset(consts[:], 0.0)
        nc.vector.memset(consts[:, 0:1], EPS)
        eps_ap = consts[:, 0:1]
        nc.sync.dma_start(id_bf[:], d_idb[:])
        nc.sync.dma_start(id_f[:], d_idf[:])
        nc.sync.dma_start(b_qkv[:], d_bqkv[:])
        nc.sync.dma_start(b_o[:], d_bo[:])
        nc.sync.dma_start(b_fc[:], d_bfc[:])
        nc.sync.dma_start(b_pj[:], d_bpj[:])
        ln1_wb = ln2_wb = None
        if use_ln_affine:
            ln1_wb = small.tile([P, 2, C], F32, tag="ln1_wb")
            ln2_wb = small.tile([P, 2, C], F32, tag="ln2_wb")
            nc.sync.dma_start(ln1_wb[:], d_ln1[:])
            nc.sync.dma_start(ln2_wb[:], d_ln2[:])
        if use_vb:
            bv_rep = small.tile([P, C], F32, tag="bv_rep")
            nc.sync.dma_start(bv_rep[:], d_bvr[:])
        if use_mask:
            mask_sb = small.tile([P, NQ, T], F32, tag="mask")
            for tt in range(NQ):
                nc.sync.dma_start(mask_sb[:, tt, :],
                                  d_mask[tt * P:(tt + 1) * P, :])

        def ln_tile(pool, src_ap, dst_bf_ap, wb):
            """LayerNorm one [128, C] fp32 tile -> bf16 dst."""
            stats = pool.tile([P, 2, 6], F32, tag="ln_stats")
            mv = pool.tile([P, 2], F32, tag="ln_mv")
            for g in range(2):
                nc.vector.bn_stats(stats[:, g, :],
                                   src_ap[:, g * 512:(g + 1) * 512])
            nc.vector.bn_aggr(mv[:], stats[:])
            std = pool.tile([P, 1], F32, tag="ln_std")
            nc.scalar.activation(std[:], mv[:, 1:2], AF.Sqrt, bias=eps_ap)
            rs = pool.tile([P, 1], F32, tag="ln_rs")
            nc.vector.reciprocal(rs[:], std[:])
            if wb is None:
                # (x - mu) * rs == x*rs + (-mu*rs): do the wide pass on ACT
                nbias = pool.tile([P, 1], F32, tag="ln_nb")
                nc.vector.tensor_scalar(nbias[:], mv[:, 0:1], rs[:], -1.0,
                                        op0=ALU.mult, op1=ALU.mult)
                nc.scalar.activation(dst_bf_ap, src_ap, AF.Identity,
                                     bias=nbias[:], scale=rs[:])
            else:
                tmp = pool.tile([P, C], F32, tag="ln_tmp")
                nc.vector.tensor_scalar(tmp[:], src_ap, mv[:, 0:1], rs[:],
                                        op0=ALU.subtract, op1=ALU.mult)
                nc.vector.scalar_tensor_tensor(tmp[:], tmp[:], 1.0,
                                               wb[:, 0, :],
                                               op0=ALU.mult, op1=ALU.mult)
                nc.vector.tensor_add(dst_bf_ap, tmp[:], wb[:, 1, :])

        def transpose_bf(src_ap, dst_tile, dst_ci_base, dst_col0, n, eng):
            """PE-transpose n [128,128] bf16 blocks of src_ap (free offset
            i*128) into dst_tile[:, dst_ci_base+i, dst_col0:+128].
            Groups of 4 share one PSUM bank and one copy instruction."""
            for g0 in range(0, n, 4):
                gn = min(4, n - g0)
                pst = ps_pool.tile([P, 4 * P], BF16, tag="ps")
                for i in range(gn):
                    nc.tensor.transpose(
                        pst[:, i * P:(i + 1) * P],
                        src_ap[:, (g0 + i) * P:(g0 + i + 1) * P], id_bf[:])
                dst = dst_tile[:, dst_ci_base + g0:dst_ci_base + g0 + gn,
                               dst_col0:dst_col0 + P]
                src = pst[:, 0:gn * P].rearrange("p (g f) -> p g f", g=gn)
                if eng == "v":
                    nc.vector.tensor_copy(dst, src)
                else:
                    nc.scalar.copy(dst, src)

        # ================= Phase A: LN1 + transposes ====================
        with tc.tile_pool(name="phA", bufs=5) as pa:
            # xq first so the q projection can start while the rest of
            # phase A is still running
            for tt in range(NQ):
                xt = pa.tile([P, C], F32, tag="x_t")
                nc.sync.dma_start(xt[:], d_xq[tt * P:(tt + 1) * P, :])
                hqb = pa.tile([P, C], BF16, tag="h_b")
                ln_tile(pa, xt[:], hqb[:], ln1_wb)
                transpose_bf(hqb[:], hqT, 0, tt * P, 4, "v")
                transpose_bf(hqb[:, 4 * P:], hqT, 4, tt * P, 4, "s")
            for ti in range(NT):
                xt = pa.tile([P, C], F32, tag="x_t")
                nc.sync.dma_start(xt[:], d_x[ti * P:(ti + 1) * P, :])
                hb = pa.tile([P, C], BF16, tag="h_b")
                ln_tile(pa, xt[:], hb[:], ln1_wb)
                transpose_bf(hb[:], hT, 0, ti * P, 4, "v")
                transpose_bf(hb[:, 4 * P:], hT, 4, ti * P, 4, "s")

        # ================= Phase B: QKV projections =====================
        with tc.tile_pool(name="phB", bufs=5) as pb, \
             tc.tile_pool(name="phBv", bufs=1) as pbv:
            for m in range(NCH):
                wsb = pb.tile([P, C], BF16, tag="w_qk")
                nc.sync.dma_start(wsb[:], d_wq[m])
                ps = ps_pool.tile([P, TQ], F32, tag="ps")
                for kt in range(NCH):
                    nc.tensor.matmul(ps[:], wsb[:, kt * P:(kt + 1) * P],
                                     hqT[:, kt, :],
                                     start=(kt == 0), stop=(kt == NCH - 1))
                nc.scalar.activation(qT[:, m, :], ps[:], AF.Identity,
                                     bias=b_qkv[:, m:m + 1])
            for m in range(NCH):
                wsb = pb.tile([P, C], BF16, tag="w_qk")
                nc.sync.dma_start(wsb[:], d_wk[m])
                for nh in range(2):
                    ps = ps_pool.tile([P, 512], F32, tag="ps")
                    for kt in range(NCH):
                        nc.tensor.matmul(
                            ps[:], wsb[:, kt * P:(kt + 1) * P],
                            hT[:, kt, nh * 512:(nh + 1) * 512],
                            start=(kt == 0), stop=(kt == NCH - 1))
                    dst = kT[:, m, nh * 512:(nh + 1) * 512]
                    if (m + nh) % 2:
                        nc.scalar.activation(
                            dst, ps[:], AF.Identity,
                            bias=b_qkv[:, NCH + m:NCH + m + 1])
                    else:
                        nc.vector.tensor_scalar_add(
                            dst, ps[:], b_qkv[:, NCH + m:NCH + m + 1])
            wv_sb = pbv.tile([P, NCH, C], BF16, tag="w_v")
            for kt in range(NCH):
                nc.sync.dma_start(wv_sb[:, kt, :], d_wv[kt])
            for ti in range(NT):
                for nh in range(2):
                    ps = ps_pool.tile([P, 512], F32, tag="ps")
                    for kt in range(NCH):
                        nc.tensor.matmul(
                            ps[:], hT[:, kt, ti * P:(ti + 1) * P],
                            wv_sb[:, kt, nh * 512:(nh + 1) * 512],
                            start=(kt == 0), stop=(kt == NCH - 1))
                    dst = vtok[:, ti, nh * 512:(nh + 1) * 512]
                    if use_vb:
                        nc.vector.tensor_add(
                            dst, ps[:], bv_rep[:, nh * 512:(nh + 1) * 512])
                    elif (ti + nh) % 2:
                        nc.scalar.copy(dst, ps[:])
                    else:
                        nc.vector.tensor_copy(dst, ps[:])

        # ================= Phase C: attention ===========================
        # w_o weights prefetched here so phase D's matmuls are not stuck
        # behind the attention-output DMA backlog.
        es_wo = ExitStack()
        pdw = es_wo.enter_context(tc.tile_pool(name="phDw", bufs=NCH))
        wo_sb = []
        for m in range(NCH):
            w = pdw.tile([P, C], BF16, tag="w_o", name=f"w_o_{m}")
            nc.sync.dma_start(w[:], d_wo[m])
            wo_sb.append(w)

        with tc.tile_pool(name="phC", bufs=3) as pc_, \
             tc.tile_pool(name="phCt", bufs=2) as pct:
            for h in range(H):
                fi, po = h // 2, (h % 2) * HD
                attT = pct.tile([P, NCH, TQ], BF16, tag="attT")
                for qt in range(NQ):
                    ssc = sc_pool.tile([P, T], F32, tag="sc")
                    for nh in range(2):
                        nc.tensor.matmul(
                            ssc[:, nh * 512:(nh + 1) * 512],
                            qT[po:po + HD, fi, qt * P:(qt + 1) * P],
                            kT[po:po + HD, fi, nh * 512:(nh + 1) * 512],
                            start=True, stop=True)
                    ex = pc_.tile([P, T], F32, tag="ex")
                    den = pc_.tile([P, 1], F32, tag="den")
                    if use_mask:
                        exin = pc_.tile([P, T], F32, tag="exin")
                        nc.vector.tensor_add(exin[:], ssc[:],
                                             mask_sb[:, qt, :])
                        nc.scalar.activation(ex[:], exin[:], AF.Exp,
                                             scale=SM_SCALE,
                                             accum_out=den[:])
                    else:
                        nc.scalar.activation(ex[:], ssc[:], AF.Exp,
                                             scale=SM_SCALE,
                                             accum_out=den[:])
                    rec = pc_.tile([P, 1], F32, tag="rec")
                    nc.vector.reciprocal(rec[:], den[:])
                    att = pc_.tile([P, T], F32, tag="att")
                    nc.vector.tensor_scalar_mul(att[:], ex[:], rec[:])
                    nc.sync.dma_start(d_att[h, qt * P:(qt + 1) * P, :],
                                      att[:])
                    for g0 in range(0, NCH, 4):
                        pst = ps_pool.tile([P, 512], F32, tag="ps")
                        for i in range(4):
                            nc.tensor.transpose(
                                pst[:, i * P:(i + 1) * P],
                                att[:, (g0 + i) * P:(g0 + i + 1) * P],
                                id_f[:])
                        dst = attT[:, g0:g0 + 4, qt * P:(qt + 1) * P]
                        src = pst[:].rearrange("p (g f) -> p g f", g=4)
                        if (qt + g0 // 4) % 2:
                            nc.scalar.copy(dst, src)
                        else:
                            nc.vector.tensor_copy(dst, src)
                ps_y = ps_pool.tile([HD, TQ], F32, tag="ps")
                v0 = fi * P + po
                for kt in range(NCH):
                    nc.tensor.matmul(ps_y[:], vtok[:, kt, v0:v0 + HD],
                                     attT[:, kt, :],
                                     start=(kt == 0), stop=(kt == NCH - 1))
                nc.vector.tensor_copy(yT[po:po + HD, fi, :], ps_y[:])

        # ============ Phase D: output proj + residual + LN2 =============
        with tc.tile_pool(name="phD", bufs=3) as pd_, \
             tc.tile_pool(name="phDao", bufs=1) as pdao:
            aoT = pdao.tile([P, NCH, TQ], BF16, tag="aoT")
            for m in range(NCH):
                wsb = wo_sb[m]
                ps = ps_pool.tile([P, TQ], F32, tag="ps")
                for kt in range(NCH):
                    nc.tensor.matmul(ps[:], wsb[:, kt * P:(kt + 1) * P],
                                     yT[:, kt, :],
                                     start=(kt == 0), stop=(kt == NCH - 1))
                nc.scalar.activation(aoT[:, m, :], ps[:], AF.Identity,
                                     bias=b_o[:, m:m + 1])
            for tt in range(NQ):
                xqt = pd_.tile([P, C], F32, tag="xq_t")
                nc.sync.dma_start(xqt[:], d_xq[tt * P:(tt + 1) * P, :])
                pst = ps_pool.tile([P, C], BF16, tag="ps")
                for m in range(NCH):
                    nc.tensor.transpose(pst[:, m * P:(m + 1) * P],
                                        aoT[:, m, tt * P:(tt + 1) * P],
                                        id_bf[:])
                nc.vector.tensor_add(xmid[:, tt, :], pst[:], xqt[:])
                h2b = pd_.tile([P, C], BF16, tag="h2_b")
                ln_tile(pd_, xmid[:, tt, :], h2b[:], ln2_wb)
                transpose_bf(h2b[:], h2T, 0, tt * P, 4, "v")
                transpose_bf(h2b[:, 4 * P:], h2T, 4, tt * P, 4, "s")
        es_wo.close()

        # ================= Phase E: FFN =================================
        with tc.tile_pool(name="phE", bufs=3) as pe, \
             tc.tile_pool(name="phEw", bufs=6) as pew:
            for m in range(NF):
                wsb = pew.tile([P, C], BF16, tag="w_fc")
                nc.sync.dma_start(wsb[:], d_wfc[m])
                ps = ps_pool.tile([P, TQ], F32, tag="ps")
                for kt in range(NCH):
                    nc.tensor.matmul(ps[:], wsb[:, kt * P:(kt + 1) * P],
                                     h2T[:, kt, :],
                                     start=(kt == 0), stop=(kt == NCH - 1))
                nc.scalar.activation(gT[:, m, :], ps[:], AF.Gelu,
                                     bias=b_fc[:, m:m + 1])
            for m in range(NCH):
                wsb = pe.tile([P, 4 * C], BF16, tag="w_pj")
                nc.sync.dma_start(wsb[:], d_wpj[m])
                ps = ps_pool.tile([P, TQ], F32, tag="ps")
                for kt in range(NF):
                    nc.tensor.matmul(ps[:], wsb[:, kt * P:(kt + 1) * P],
                                     gT[:, kt, :],
                                     start=(kt == 0), stop=(kt == NF - 1))
                nc.scalar.activation(pT[:, m, :], ps[:], AF.Identity,
                                     bias=b_pj[:, m:m + 1])
            for tt in range(NQ):
                pst = ps_pool.tile([P, C], BF16, tag="ps")
                for m in range(NCH):
                    nc.tensor.transpose(pst[:, m * P:(m + 1) * P],
                                        pT[:, m, tt * P:(tt + 1) * P],
                                        id_bf[:])
                xo = pe.tile([P, C], F32, tag="xo")
                nc.vector.tensor_add(xo[:], pst[:], xmid[:, tt, :])
                nc.sync.dma_start(d_xo[tt * P:(tt + 1) * P, :], xo[:])

    nc.compile()
    return nc


_NC_CACHE = {}
_LAST_IN_MAPS = None


def _get_nc(key):
    if key not in _NC_CACHE:
        _NC_CACHE[key] = build_nc(*key)
    return _NC_CACHE[key]


def kernel(x, attention_mask, ln1_w, ln1_b, w_qkv, b_qkv, w_o, b_o,
           ln2_w, ln2_b, w_fc, b_fc, w_proj, b_proj):
    x = np.asarray(x, np.float32)
    attention_mask = np.asarray(attention_mask, np.float32)
    ln1_w = np.asarray(ln1_w, np.float32)
    ln1_b = np.asarray(ln1_b, np.float32)
    w_qkv = np.asarray(w_qkv, np.float32)
    b_qkv_a = np.asarray(b_qkv, np.float32)
    w_o = np.asarray(w_o, np.float32)
    b_o_a = np.asarray(b_o, np.float32)
    ln2_w = np.asarray(ln2_w, np.float32)
    ln2_b = np.asarray(ln2_b, np.float32)
    w_fc = np.asarray(w_fc, np.float32)
    b_fc_a = np.asarray(b_fc, np.float32)
    w_proj = np.asarray(w_proj, np.float32)
    b_proj_a = np.asarray(b_proj, np.float32)

    use_mask = bool(np.any(attention_mask))
    use_ln_affine = not (np.all(ln1_w == 1) and np.all(ln1_b == 0)
                         and np.all(ln2_w == 1) and np.all(ln2_b == 0))
    use_vb = bool(np.any(b_qkv_a[2 * C:]))

    nc = _get_nc((use_mask, use_ln_affine, use_vb))

    shared = {
        "wq4": _w4(w_qkv[0:C]),
        "wk4": _w4(w_qkv[C:2 * C]),
        "wvr": np.ascontiguousarray(
            w_qkv[2 * C:3 * C].T.reshape(NCH, P, C)).astype(_BF),
        "wo4": _w4(w_o),
        "wfc4": _w4(w_fc),
        "wpj4": _w4(w_proj),
        "bqkv": np.ascontiguousarray(b_qkv_a.reshape(3 * NCH, P).T),
        "bo": np.ascontiguousarray(b_o_a.reshape(NCH, P).T),
        "bfc": np.ascontiguousarray(b_fc_a.reshape(NF, P).T),
        "bpj": np.ascontiguousarray(b_proj_a.reshape(NCH, P).T),
        "ident_bf": np.eye(P, dtype=_BF),
        "ident_f": np.eye(P, dtype=np.float32),
    }
    if use_ln_affine:
        shared["ln1_wb"] = np.ascontiguousarray(np.broadcast_to(
            np.stack([ln1_w, ln1_b]), (P, 2, C)))
        shared["ln2_wb"] = np.ascontiguousarray(np.broadcast_to(
            np.stack([ln2_w, ln2_b]), (P, 2, C)))
    if use_vb:
        shared["bv_rep"] = np.ascontiguousarray(
            np.broadcast_to(b_qkv_a[2 * C:], (P, C)))

    in_maps = []
    for core in range(8):
        b, qh = core // 2, core % 2
        m = dict(shared)
        m["x_full"] = np.ascontiguousarray(x[b])
        m["xq"] = np.ascontiguousarray(x[b, qh * TQ:(qh + 1) * TQ])
        if use_mask:
            # activation computes exp(scale*(s + m')), so pre-divide the
            # mask by scale to get exp(scale*s + mask).
            m["mask_q"] = np.ascontiguousarray(
                np.broadcast_to(attention_mask[0, 0], (T, T))
                [qh * TQ:(qh + 1) * TQ] / SM_SCALE)
        in_maps.append(m)

    global _LAST_IN_MAPS
    _LAST_IN_MAPS = in_maps
    res = run_bass_kernel_spmd(nc, in_maps, list(range(8)))

    x_out = np.empty((B, T, C), np.float32)
    att = np.empty((B, H, T, T), np.float32)
    for core in range(8):
        b, qh = core // 2, core % 2
        x_out[b, qh * TQ:(qh + 1) * TQ] = res.results[core]["x_out"]
        att[b, :, qh * TQ:(qh + 1) * TQ, :] = res.results[core]["att_out"]
    return (x_out, att)


# revision 33
# speedup vs baseline: 1.0890x; 1.0442x over previous
"""GPT transformer block (B=4, T=1024, C=1024, H=16) on 8 Trainium2 cores.

Sharding: core = b*2 + qh  (b = batch element, qh = query-half of 512 tokens).
Each core computes K/V for its whole batch element (duplicated across the two
cores sharing it) and everything else — attention rows, attention-weight
output slice, MLP — for its own 512 query tokens.  No collectives; the host
concatenates the 8 output slices.

Matmuls run in bf16 with fp32 PSUM accumulation; layernorm, softmax and both
residual adds stay in fp32.
"""

from contextlib import ExitStack

import numpy as np
import ml_dtypes

import concourse.bacc as bacc
import concourse.mybir as mybir
from concourse.tile import TileContext
from concourse.bass_utils import run_bass_kernel_spmd

F32 = mybir.dt.float32
BF16 = mybir.dt.bfloat16
AF = mybir.ActivationFunctionType
ALU = mybir.AluOpType

B, T, C, H = 4, 1024, 1024, 16
HD = C // H          # 64
TQ = T // 2          # 512 query tokens per core
P = 128
NT = T // P          # 8 token tiles (full)
NQ = TQ // P         # 4 query token tiles
NCH = C // P         # 8 channel tiles
NF = 4 * C // P      # 32 ffn hidden tiles
EPS = 1e-5
SM_SCALE = 1.0 / np.sqrt(HD)

_BF = ml_dtypes.bfloat16


def _w4(w):
    """[M,K] weight -> [M/128, 128, K] bf16 lhsT-tile layout.

    out[m, p, kt*128+j] = w[m*128+j, kt*128+p]  (= w.T[kt*128+p, m*128+j])
    """
    M, K = w.shape
    nm, nk = M // P, K // P
    return np.ascontiguousarray(
        w.reshape(nm, P, nk, P).transpose(0, 3, 2, 1)).astype(_BF).reshape(
            nm, P, K)


def build_nc(use_mask: bool, use_ln_affine: bool, use_vb: bool):
    nc = bacc.Bacc(target_bir_lowering=False, num_swdge_queues=4)

    d_x = nc.declare_dram_parameter("x_full", [T, C], F32, isOutput=False)
    d_xq = nc.declare_dram_parameter("xq", [TQ, C], F32, isOutput=False)
    d_wq = nc.declare_dram_parameter("wq4", [NCH, P, C], BF16, isOutput=False)
    d_wk = nc.declare_dram_parameter("wk4", [NCH, P, C], BF16, isOutput=False)
    d_wv = nc.declare_dram_parameter("wvr", [NCH, P, C], BF16, isOutput=False)
    d_wo = nc.declare_dram_parameter("wo4", [NCH, P, C], BF16, isOutput=False)
    d_wfc = nc.declare_dram_parameter("wfc4", [NF, P, C], BF16, isOutput=False)
    d_wpj = nc.declare_dram_parameter("wpj4", [NCH, P, 4 * C], BF16,
                                      isOutput=False)
    d_bqkv = nc.declare_dram_parameter("bqkv", [P, 3 * NCH], F32, isOutput=False)
    d_bo = nc.declare_dram_parameter("bo", [P, NCH], F32, isOutput=False)
    d_bfc = nc.declare_dram_parameter("bfc", [P, NF], F32, isOutput=False)
    d_bpj = nc.declare_dram_parameter("bpj", [P, NCH], F32, isOutput=False)
    d_idb = nc.declare_dram_parameter("ident_bf", [P, P], BF16, isOutput=False)
    d_idf = nc.declare_dram_parameter("ident_f", [P, P], F32, isOutput=False)
    if use_mask:
        d_mask = nc.declare_dram_parameter("mask_q", [TQ, T], F32,
                                           isOutput=False)
    if use_ln_affine:
        d_ln1 = nc.declare_dram_parameter("ln1_wb", [P, 2, C], F32,
                                          isOutput=False)
        d_ln2 = nc.declare_dram_parameter("ln2_wb", [P, 2, C], F32,
                                          isOutput=False)
    if use_vb:
        d_bvr = nc.declare_dram_parameter("bv_rep", [P, C], F32, isOutput=False)

    d_att = nc.declare_dram_parameter("att_out", [H, TQ, T], F32, isOutput=True)
    d_xo = nc.declare_dram_parameter("x_out", [TQ, C], F32, isOutput=True)

    with TileContext(nc) as tc, ExitStack() as top:
        small = top.enter_context(tc.tile_pool(name="small", bufs=1))
        big = top.enter_context(tc.tile_pool(name="big", bufs=1))
        ps_pool = top.enter_context(tc.tile_pool(name="ps", bufs=4,
                                                 space="PSUM"))
        sc_pool = top.enter_context(tc.tile_pool(name="sc", bufs=2,
                                                 space="PSUM"))

        consts = small.tile([P, 2], F32, tag="consts")
        id_bf = small.tile([P, P], BF16, tag="id_bf")
        id_f = small.tile([P, P], F32, tag="id_f")
        b_qkv = small.tile([P, 3 * NCH], F32, tag="b_qkv")
        b_o = small.tile([P, NCH], F32, tag="b_o")
        b_fc = small.tile([P, NF], F32, tag="b_fc")
        b_pj = small.tile([P, NCH], F32, tag="b_pj")

        # cross-phase tensors; disjoint lifetimes share a slot via the tag
        hT = big.tile([P, NT, T], BF16, tag="s_hT_gT")     # 16K  A..B
        gT = big.tile([P, NF, TQ], BF16, tag="s_hT_gT")    # 32K  E
        hqT = big.tile([P, NCH, TQ], BF16, tag="s_hqT_yT")  # 8K  A..B
        yT = big.tile([P, NCH, TQ], BF16, tag="s_hqT_yT")   # 8K  C..D
        kT = big.tile([P, NCH, T], BF16, tag="s_kT_xmid")  # 16K  B..C
        xmids = [big.tile([P, C], F32, tag=f"xmid{i}", name=f"xmid{i}")
                 for i in range(NQ)]                       # 16K  D..E
        qT = big.tile([P, NCH, TQ], BF16, tag="s_qT_h2T")   # 8K  B..C
        h2T = big.tile([P, NCH, TQ], BF16, tag="s_qT_h2T")  # 8K  D..E
        vtok = big.tile([P, NT, C], BF16, tag="s_vtok_pT")  # 16K B..C
        pT = big.tile([P, NCH, TQ], BF16, tag="s_vtok_pT")  # 8K  E

        nc.vector.memset(consts[:], 0.0)
        nc.vector.memset(consts[:, 0:1], EPS)
        eps_ap = consts[:, 0:1]
        nc.sync.dma_start(id_bf[:], d_idb[:])
        nc.sync.dma_start(id_f[:], d_idf[:])
        # biases etc. aren't needed until phase B — issue on gpsimd's
        # software DGE so the sync-engine queue stays free for x/xq.
        nc.gpsimd.dma_start(b_qkv[:], d_bqkv[:])
        nc.gpsimd.dma_start(b_o[:], d_bo[:])
        nc.gpsimd.dma_start(b_fc[:], d_bfc[:])
        nc.gpsimd.dma_start(b_pj[:], d_bpj[:])
        ln1_wb = ln2_wb = None
        if use_ln_affine:
            ln1_wb = small.tile([P, 2, C], F32, tag="ln1_wb")
            ln2_wb = small.tile([P, 2, C], F32, tag="ln2_wb")
            nc.gpsimd.dma_start(ln1_wb[:], d_ln1[:])
            nc.gpsimd.dma_start(ln2_wb[:], d_ln2[:])
        if use_vb:
            bv_rep = small.tile([P, C], F32, tag="bv_rep")
            nc.gpsimd.dma_start(bv_rep[:], d_bvr[:])
        if use_mask:
            mask_sb = small.tile([P, NQ, T], F32, tag="mask")
            for tt in range(NQ):
                nc.gpsimd.dma_start(mask_sb[:, tt, :],
                                    d_mask[tt * P:(tt + 1) * P, :])

        def ln_tile(pool, src_ap, dst_bf_ap, wb, idx=0):
            """LayerNorm one [128, C] fp32 tile -> bf16 dst.

            The wide affine pass alternates between DVE and ACT by idx so
            neither engine serializes the tile pipeline.
            """
            stats = pool.tile([P, 2, 6], F32, tag="ln_stats")
            mv = pool.tile([P, 2], F32, tag="ln_mv")
            for g in range(2):
                nc.vector.bn_stats(stats[:, g, :],
                                   src_ap[:, g * 512:(g + 1) * 512])
            nc.vector.bn_aggr(mv[:], stats[:])
            std = pool.tile([P, 1], F32, tag="ln_std")
            nc.scalar.activation(std[:], mv[:, 1:2], AF.Sqrt, bias=eps_ap)
            rs = pool.tile([P, 1], F32, tag="ln_rs")
            nc.vector.reciprocal(rs[:], std[:])
            if wb is None:
                if idx % 2:
                    nc.vector.tensor_scalar(dst_bf_ap, src_ap, mv[:, 0:1],
                                            rs[:], op0=ALU.subtract,
                                            op1=ALU.mult)
                else:
                    # (x - mu) * rs == x*rs + (-mu*rs): wide pass on ACT
                    nbias = pool.tile([P, 1], F32, tag="ln_nb")
                    nc.vector.tensor_scalar(nbias[:], mv[:, 0:1], rs[:], -1.0,
                                            op0=ALU.mult, op1=ALU.mult)
                    nc.scalar.activation(dst_bf_ap, src_ap, AF.Identity,
                                         bias=nbias[:], scale=rs[:])
            else:
                tmp = pool.tile([P, C], F32, tag="ln_tmp")
                nc.vector.tensor_scalar(tmp[:], src_ap, mv[:, 0:1], rs[:],
                                        op0=ALU.subtract, op1=ALU.mult)
                nc.vector.scalar_tensor_tensor(tmp[:], tmp[:], 1.0,
                                               wb[:, 0, :],
                                               op0=ALU.mult, op1=ALU.mult)
                nc.vector.tensor_add(dst_bf_ap, tmp[:], wb[:, 1, :])

        def transpose_bf(src_ap, dst_tile, dst_ci_base, dst_col0, n, eng):
            """PE-transpose n [128,128] bf16 blocks of src_ap (free offset
            i*128) into dst_tile[:, dst_ci_base+i, dst_col0:+128].
            Groups of 4 share one PSUM bank and one copy instruction."""
            for g0 in range(0, n, 4):
                gn = min(4, n - g0)
                pst = ps_pool.tile([P, 4 * P], BF16, tag="ps")
                for i in range(gn):
                    nc.tensor.transpose(
                        pst[:, i * P:(i + 1) * P],
                        src_ap[:, (g0 + i) * P:(g0 + i + 1) * P], id_bf[:])
                dst = dst_tile[:, dst_ci_base + g0:dst_ci_base + g0 + gn,
                               dst_col0:dst_col0 + P]
                src = pst[:, 0:gn * P].rearrange("p (g f) -> p g f", g=gn)
                if eng == "v":
                    nc.vector.tensor_copy(dst, src)
                else:
                    nc.scalar.copy(dst, src)

        # ================= Phase A: LN1 + transposes ====================
        with tc.tile_pool(name="phA", bufs=5) as pa:
            # xq first so the q projection can start while the rest of
            # phase A is still running
            for tt in range(NQ):
                xt = pa.tile([P, C], F32, tag="x_t")
                nc.sync.dma_start(xt[:], d_xq[tt * P:(tt + 1) * P, :])
                hqb = pa.tile([P, C], BF16, tag="h_b")
                ln_tile(pa, xt[:], hqb[:], ln1_wb, tt)
                transpose_bf(hqb[:], hqT, 0, tt * P, 4, "v")
                transpose_bf(hqb[:, 4 * P:], hqT, 4, tt * P, 4, "s")
            for ti in range(NT):
                xt = pa.tile([P, C], F32, tag="x_t")
                nc.sync.dma_start(xt[:], d_x[ti * P:(ti + 1) * P, :])
                hb = pa.tile([P, C], BF16, tag="h_b")
                ln_tile(pa, xt[:], hb[:], ln1_wb, ti)
                transpose_bf(hb[:], hT, 0, ti * P, 4, "v")
                transpose_bf(hb[:, 4 * P:], hT, 4, ti * P, 4, "s")

        # ================= Phase B: QKV projections =====================
        with tc.tile_pool(name="phB", bufs=5) as pb, \
             tc.tile_pool(name="phBv", bufs=1) as pbv:
            for m in range(NCH):
                wsb = pb.tile([P, C], BF16, tag="w_qk")
                nc.sync.dma_start(wsb[:], d_wq[m])
                ps = ps_pool.tile([P, TQ], F32, tag="ps")
                for kt in range(NCH):
                    nc.tensor.matmul(ps[:], wsb[:, kt * P:(kt + 1) * P],
                                     hqT[:, kt, :],
                                     start=(kt == 0), stop=(kt == NCH - 1))
                nc.scalar.activation(qT[:, m, :], ps[:], AF.Identity,
                                     bias=b_qkv[:, m:m + 1])
            for m in range(NCH):
                wsb = pb.tile([P, C], BF16, tag="w_qk")
                nc.sync.dma_start(wsb[:], d_wk[m])
                for nh in range(2):
                    ps = ps_pool.tile([P, 512], F32, tag="ps")
                    for kt in range(NCH):
                        nc.tensor.matmul(
                            ps[:], wsb[:, kt * P:(kt + 1) * P],
                            hT[:, kt, nh * 512:(nh + 1) * 512],
                            start=(kt == 0), stop=(kt == NCH - 1))
                    dst = kT[:, m, nh * 512:(nh + 1) * 512]
                    if (m + nh) % 2:
                        nc.scalar.activation(
                            dst, ps[:], AF.Identity,
                            bias=b_qkv[:, NCH + m:NCH + m + 1])
                    else:
                        nc.vector.tensor_scalar_add(
                            dst, ps[:], b_qkv[:, NCH + m:NCH + m + 1])
            wv_sb = pbv.tile([P, NCH, C], BF16, tag="w_v")
            for kt in range(NCH):
                nc.sync.dma_start(wv_sb[:, kt, :], d_wv[kt])
            for ti in range(NT):
                for nh in range(2):
                    ps = ps_pool.tile([P, 512], F32, tag="ps")
                    for kt in range(NCH):
                        nc.tensor.matmul(
                            ps[:], hT[:, kt, ti * P:(ti + 1) * P],
                            wv_sb[:, kt, nh * 512:(nh + 1) * 512],
                            start=(kt == 0), stop=(kt == NCH - 1))
                    dst = vtok[:, ti, nh * 512:(nh + 1) * 512]
                    if use_vb:
                        nc.vector.tensor_add(
                            dst, ps[:], bv_rep[:, nh * 512:(nh + 1) * 512])
                    elif (ti + nh) % 2:
                        nc.scalar.copy(dst, ps[:])
                    else:
                        nc.vector.tensor_copy(dst, ps[:])

        # ================= Phase C: attention ===========================
        # w_o weights prefetched here so phase D's matmuls are not stuck
        # behind the attention-output DMA backlog.
        es_wo = ExitStack()
        pdw = es_wo.enter_context(tc.tile_pool(name="phDw", bufs=NCH))
        wo_sb = []
        for m in range(NCH):
            w = pdw.tile([P, C], BF16, tag="w_o", name=f"w_o_{m}")
            nc.sync.dma_start(w[:], d_wo[m])
            wo_sb.append(w)

        with tc.tile_pool(name="phC", bufs=3) as pc_, \
             tc.tile_pool(name="phCt", bufs=2) as pct:
            for h in range(H):
                fi, po = h // 2, (h % 2) * HD
                attT = pct.tile([P, NCH, TQ], BF16, tag="attT")
                for qt in range(NQ):
                    ssc = sc_pool.tile([P, T], F32, tag="sc")
                    for nh in range(2):
                        nc.tensor.matmul(
                            ssc[:, nh * 512:(nh + 1) * 512],
                            qT[po:po + HD, fi, qt * P:(qt + 1) * P],
                            kT[po:po + HD, fi, nh * 512:(nh + 1) * 512],
                            start=True, stop=True)
                    ex = pc_.tile([P, T], F32, tag="ex")
                    den = pc_.tile([P, 1], F32, tag="den")
                    if use_mask:
                        exin = pc_.tile([P, T], F32, tag="exin")
                        nc.vector.tensor_add(exin[:], ssc[:],
                                             mask_sb[:, qt, :])
                        nc.scalar.activation(ex[:], exin[:], AF.Exp,
                                             scale=SM_SCALE,
                                             accum_out=den[:])
                    else:
                        nc.scalar.activation(ex[:], ssc[:], AF.Exp,
                                             scale=SM_SCALE,
                                             accum_out=den[:])
                    rec = pc_.tile([P, 1], F32, tag="rec")
                    nc.vector.reciprocal(rec[:], den[:])
                    att = pc_.tile([P, T], F32, tag="att")
                    nc.vector.tensor_scalar_mul(att[:], ex[:], rec[:])
                    nc.gpsimd.dma_start(d_att[h, qt * P:(qt + 1) * P, :],
                                        att[:])
                    for g0 in range(0, NCH, 4):
                        pst = ps_pool.tile([P, 512], F32, tag="ps")
                        for i in range(4):
                            nc.tensor.transpose(
                                pst[:, i * P:(i + 1) * P],
                                att[:, (g0 + i) * P:(g0 + i + 1) * P],
                                id_f[:])
                        dst = attT[:, g0:g0 + 4, qt * P:(qt + 1) * P]
                        src = pst[:].rearrange("p (g f) -> p g f", g=4)
                        if (qt + g0 // 4) % 2:
                            nc.scalar.copy(dst, src)
                        else:
                            nc.vector.tensor_copy(dst, src)
                ps_y = ps_pool.tile([HD, TQ], F32, tag="ps")
                v0 = fi * P + po
                for kt in range(NCH):
                    nc.tensor.matmul(ps_y[:], vtok[:, kt, v0:v0 + HD],
                                     attT[:, kt, :],
                                     start=(kt == 0), stop=(kt == NCH - 1))
                nc.vector.tensor_copy(yT[po:po + HD, fi, :], ps_y[:])

        # ============ Phase D: output proj + residual + LN2 =============
        with tc.tile_pool(name="phD", bufs=3) as pd_, \
             tc.tile_pool(name="phDao", bufs=1) as pdao:
            aoT = pdao.tile([P, NCH, TQ], BF16, tag="aoT")
            for m in range(NCH):
                wsb = wo_sb[m]
                ps = ps_pool.tile([P, TQ], F32, tag="ps")
                for kt in range(NCH):
                    nc.tensor.matmul(ps[:], wsb[:, kt * P:(kt + 1) * P],
                                     yT[:, kt, :],
                                     start=(kt == 0), stop=(kt == NCH - 1))
                nc.scalar.activation(aoT[:, m, :], ps[:], AF.Identity,
                                     bias=b_o[:, m:m + 1])
            for tt in range(NQ):
                xqt = pd_.tile([P, C], F32, tag="xq_t")
                nc.sync.dma_start(xqt[:], d_xq[tt * P:(tt + 1) * P, :])
                pst = ps_pool.tile([P, C], BF16, tag="ps")
                for m in range(NCH):
                    nc.tensor.transpose(pst[:, m * P:(m + 1) * P],
                                        aoT[:, m, tt * P:(tt + 1) * P],
                                        id_bf[:])
                nc.vector.tensor_add(xmids[tt][:], pst[:], xqt[:])
                h2b = pd_.tile([P, C], BF16, tag="h2_b")
                ln_tile(pd_, xmids[tt][:], h2b[:], ln2_wb, tt)
                transpose_bf(h2b[:], h2T, 0, tt * P, 4, "v")
                transpose_bf(h2b[:, 4 * P:], h2T, 4, tt * P, 4, "s")
        es_wo.close()

        # ================= Phase E: FFN =================================
        with tc.tile_pool(name="phE", bufs=3) as pe, \
             tc.tile_pool(name="phEw", bufs=6) as pew:
            for m in range(NF):
                wsb = pew.tile([P, C], BF16, tag="w_fc")
                nc.sync.dma_start(wsb[:], d_wfc[m])
                ps = ps_pool.tile([P, TQ], F32, tag="ps")
                for kt in range(NCH):
                    nc.tensor.matmul(ps[:], wsb[:, kt * P:(kt + 1) * P],
                                     h2T[:, kt, :],
                                     start=(kt == 0), stop=(kt == NCH - 1))
                nc.scalar.activation(gT[:, m, :], ps[:], AF.Gelu,
                                     bias=b_fc[:, m:m + 1])
            for m in range(NCH):
                wsb = pe.tile([P, 4 * C], BF16, tag="w_pj")
                nc.sync.dma_start(wsb[:], d_wpj[m])
                ps = ps_pool.tile([P, TQ], F32, tag="ps")
                for kt in range(NF):
                    nc.tensor.matmul(ps[:], wsb[:, kt * P:(kt + 1) * P],
                                     gT[:, kt, :],
                                     start=(kt == 0), stop=(kt == NF - 1))
                nc.scalar.activation(pT[:, m, :], ps[:], AF.Identity,
                                     bias=b_pj[:, m:m + 1])
            for tt in range(NQ):
                pst = ps_pool.tile([P, C], BF16, tag="ps")
                for m in range(NCH):
                    nc.tensor.transpose(pst[:, m * P:(m + 1) * P],
                                        pT[:, m, tt * P:(tt + 1) * P],
                                        id_bf[:])
                xo = pe.tile([P, C], F32, tag="xo")
                nc.vector.tensor_add(xo[:], pst[:], xmids[tt][:])
                nc.sync.dma_start(d_xo[tt * P:(tt + 1) * P, :], xo[:])

    nc.compile()
    return nc


_NC_CACHE = {}
_LAST_IN_MAPS = None


def _get_nc(key):
    if key not in _NC_CACHE:
        _NC_CACHE[key] = build_nc(*key)
    return _NC_CACHE[key]


def kernel(x, attention_mask, ln1_w, ln1_b, w_qkv, b_qkv, w_o, b_o,
           ln2_w, ln2_b, w_fc, b_fc, w_proj, b_proj):
    x = np.asarray(x, np.float32)
    attention_mask = np.asarray(attention_mask, np.float32)
    ln1_w = np.asarray(ln1_w, np.float32)
    ln1_b = np.asarray(ln1_b, np.float32)
    w_qkv = np.asarray(w_qkv, np.float32)
    b_qkv_a = np.asarray(b_qkv, np.float32)
    w_o = np.asarray(w_o, np.float32)
    b_o_a = np.asarray(b_o, np.float32)
    ln2_w = np.asarray(ln2_w, np.float32)
    ln2_b = np.asarray(ln2_b, np.float32)
    w_fc = np.asarray(w_fc, np.float32)
    b_fc_a = np.asarray(b_fc, np.float32)
    w_proj = np.asarray(w_proj, np.float32)
    b_proj_a = np.asarray(b_proj, np.float32)

    use_mask = bool(np.any(attention_mask))
    use_ln_affine = not (np.all(ln1_w == 1) and np.all(ln1_b == 0)
                         and np.all(ln2_w == 1) and np.all(ln2_b == 0))
    use_vb = bool(np.any(b_qkv_a[2 * C:]))

    nc = _get_nc((use_mask, use_ln_affine, use_vb))

    shared = {
        "wq4": _w4(w_qkv[0:C]),
        "wk4": _w4(w_qkv[C:2 * C]),
        "wvr": np.ascontiguousarray(
            w_qkv[2 * C:3 * C].T.reshape(NCH, P, C)).astype(_BF),
        "wo4": _w4(w_o),
        "wfc4": _w4(w_fc),
        "wpj4": _w4(w_proj),
        "bqkv": np.ascontiguousarray(b_qkv_a.reshape(3 * NCH, P).T),
        "bo": np.ascontiguousarray(b_o_a.reshape(NCH, P).T),
        "bfc": np.ascontiguousarray(b_fc_a.reshape(NF, P).T),
        "bpj": np.ascontiguousarray(b_proj_a.reshape(NCH, P).T),
        "ident_bf": np.eye(P, dtype=_BF),
        "ident_f": np.eye(P, dtype=np.float32),
    }
    if use_ln_affine:
        shared["ln1_wb"] = np.ascontiguousarray(np.broadcast_to(
            np.stack([ln1_w, ln1_b]), (P, 2, C)))
        shared["ln2_wb"] = np.ascontiguousarray(np.broadcast_to(
            np.stack([ln2_w, ln2_b]), (P, 2, C)))
    if use_vb:
        shared["bv_rep"] = np.ascontiguousarray(
            np.broadcast_to(b_qkv_a[2 * C:], (P, C)))

    in_maps = []
    for core in range(8):
        b, qh = core // 2, core % 2
        m = dict(shared)
        m["x_full"] = np.ascontiguousarray(x[b])
        m["xq"] = np.ascontiguousarray(x[b, qh * TQ:(qh + 1) * TQ])
        if use_mask:
            # activation computes exp(scale*(s + m')), so pre-divide the
            # mask by scale to get exp(scale*s + mask).
            m["mask_q"] = np.ascontiguousarray(
                np.broadcast_to(attention_mask[0, 0], (T, T))
                [qh * TQ:(qh + 1) * TQ] / SM_SCALE)
        in_maps.append(m)

    global _LAST_IN_MAPS
    _LAST_IN_MAPS = in_maps
    res = run_bass_kernel_spmd(nc, in_maps, list(range(8)))

    x_out = np.empty((B, T, C), np.float32)
    att = np.empty((B, H, T, T), np.float32)
    for core in range(8):
        b, qh = core // 2, core % 2
        x_out[b, qh * TQ:(qh + 1) * TQ] = res.results[core]["x_out"]
        att[b, :, qh * TQ:(qh + 1) * TQ, :] = res.results[core]["att_out"]
    return (x_out, att)


# revision 35
# speedup vs baseline: 1.1248x; 1.0328x over previous
"""GPT transformer block (B=4, T=1024, C=1024, H=16) on 8 Trainium2 cores.

Sharding: core = b*2 + qh  (b = batch element, qh = query-half of 512 tokens).
Each core computes K/V for its whole batch element (duplicated across the two
cores sharing it) and everything else — attention rows, attention-weight
output slice, MLP — for its own 512 query tokens.  No collectives; the host
concatenates the 8 output slices.

Matmuls run in bf16 with fp32 PSUM accumulation; layernorm, softmax and both
residual adds stay in fp32.
"""

from contextlib import ExitStack

import numpy as np
import ml_dtypes

import concourse.bacc as bacc
import concourse.mybir as mybir
from concourse.tile import TileContext
from concourse.bass_utils import run_bass_kernel_spmd

F32 = mybir.dt.float32
BF16 = mybir.dt.bfloat16
AF = mybir.ActivationFunctionType
ALU = mybir.AluOpType

B, T, C, H = 4, 1024, 1024, 16
HD = C // H          # 64
TQ = T // 2          # 512 query tokens per core
P = 128
NT = T // P          # 8 token tiles (full)
NQ = TQ // P         # 4 query token tiles
NCH = C // P         # 8 channel tiles
NF = 4 * C // P      # 32 ffn hidden tiles
EPS = 1e-5
SM_SCALE = 1.0 / np.sqrt(HD)

_BF = ml_dtypes.bfloat16


def _w4(w):
    """[M,K] weight -> [M/128, 128, K] bf16 lhsT-tile layout.

    out[m, p, kt*128+j] = w[m*128+j, kt*128+p]  (= w.T[kt*128+p, m*128+j])
    """
    M, K = w.shape
    nm, nk = M // P, K // P
    return np.ascontiguousarray(
        w.reshape(nm, P, nk, P).transpose(0, 3, 2, 1)).astype(_BF).reshape(
            nm, P, K)


def build_nc(use_mask: bool, use_ln_affine: bool, use_vb: bool):
    nc = bacc.Bacc(target_bir_lowering=False, num_swdge_queues=4)

    d_x = nc.declare_dram_parameter("x_full", [T, C], F32, isOutput=False)
    d_xq = nc.declare_dram_parameter("xq", [TQ, C], F32, isOutput=False)
    d_wq = nc.declare_dram_parameter("wq4", [NCH, P, C], BF16, isOutput=False)
    d_wk = nc.declare_dram_parameter("wk4", [NCH, P, C], BF16, isOutput=False)
    d_wv = nc.declare_dram_parameter("wvr", [NCH, P, C], BF16, isOutput=False)
    d_wo = nc.declare_dram_parameter("wo4", [NCH, P, C], BF16, isOutput=False)
    d_wfc = nc.declare_dram_parameter("wfc4", [NF, P, C], BF16, isOutput=False)
    d_wpj = nc.declare_dram_parameter("wpj4", [NCH, P, 4 * C], BF16,
                                      isOutput=False)
    d_bqkv = nc.declare_dram_parameter("bqkv", [P, 3 * NCH], F32, isOutput=False)
    d_bo = nc.declare_dram_parameter("bo", [P, NCH], F32, isOutput=False)
    d_bfc = nc.declare_dram_parameter("bfc", [P, NF], F32, isOutput=False)
    d_bpj = nc.declare_dram_parameter("bpj", [P, NCH], F32, isOutput=False)
    d_idb = nc.declare_dram_parameter("ident_bf", [P, P], BF16, isOutput=False)
    d_idf = nc.declare_dram_parameter("ident_f", [P, P], F32, isOutput=False)
    if use_mask:
        d_mask = nc.declare_dram_parameter("mask_q", [TQ, T], F32,
                                           isOutput=False)
    if use_ln_affine:
        d_ln1 = nc.declare_dram_parameter("ln1_wb", [P, 2, C], F32,
                                          isOutput=False)
        d_ln2 = nc.declare_dram_parameter("ln2_wb", [P, 2, C], F32,
                                          isOutput=False)
    if use_vb:
        d_bvr = nc.declare_dram_parameter("bv_rep", [P, C], F32, isOutput=False)

    d_att = nc.declare_dram_parameter("att_out", [H, TQ, T], F32, isOutput=True)
    d_xo = nc.declare_dram_parameter("x_out", [TQ, C], F32, isOutput=True)

    with TileContext(nc) as tc, ExitStack() as top:
        small = top.enter_context(tc.tile_pool(name="small", bufs=1))
        big = top.enter_context(tc.tile_pool(name="big", bufs=1))
        ps_pool = top.enter_context(tc.tile_pool(name="ps", bufs=4,
                                                 space="PSUM"))
        sc_pool = top.enter_context(tc.tile_pool(name="sc", bufs=2,
                                                 space="PSUM"))

        consts = small.tile([P, 2], F32, tag="consts")
        id_bf = small.tile([P, P], BF16, tag="id_bf")
        id_f = small.tile([P, P], F32, tag="id_f")
        b_qkv = small.tile([P, 3 * NCH], F32, tag="b_qkv")
        b_o = small.tile([P, NCH], F32, tag="b_o")
        b_fc = small.tile([P, NF], F32, tag="b_fc")
        b_pj = small.tile([P, NCH], F32, tag="b_pj")

        # cross-phase tensors; disjoint lifetimes share a slot via the tag
        hT = big.tile([P, NT, T], BF16, tag="s_hT_gT")     # 16K  A..B
        gT = big.tile([P, NF, TQ], BF16, tag="s_hT_gT")    # 32K  E
        hqT = big.tile([P, NCH, TQ], BF16, tag="s_hqT_yT")  # 8K  A..B
        yT = big.tile([P, NCH, TQ], BF16, tag="s_hqT_yT")   # 8K  C..D
        kT = big.tile([P, NCH, T], BF16, tag="s_kT_xmid")  # 16K  B..C
        xmids = [big.tile([P, C], F32, tag=f"xmid{i}", name=f"xmid{i}")
                 for i in range(NQ)]                       # 16K  D..E
        qT = big.tile([P, NCH, TQ], BF16, tag="s_qT_h2T")   # 8K  B..C
        h2T = big.tile([P, NCH, TQ], BF16, tag="s_qT_h2T")  # 8K  D..E
        vtok = big.tile([P, NT, C], BF16, tag="s_vtok_pT")  # 16K B..C
        pT = big.tile([P, NCH, TQ], BF16, tag="s_vtok_pT")  # 8K  E

        nc.vector.memset(consts[:], 0.0)
        nc.vector.memset(consts[:, 0:1], EPS)
        eps_ap = consts[:, 0:1]
        nc.sync.dma_start(id_bf[:], d_idb[:])
        nc.sync.dma_start(id_f[:], d_idf[:])
        # biases etc. aren't needed until phase B — issue on gpsimd's
        # software DGE so the sync-engine queue stays free for x/xq.
        nc.gpsimd.dma_start(b_qkv[:], d_bqkv[:])
        nc.gpsimd.dma_start(b_o[:], d_bo[:])
        nc.gpsimd.dma_start(b_fc[:], d_bfc[:])
        nc.gpsimd.dma_start(b_pj[:], d_bpj[:])
        ln1_wb = ln2_wb = None
        if use_ln_affine:
            ln1_wb = small.tile([P, 2, C], F32, tag="ln1_wb")
            ln2_wb = small.tile([P, 2, C], F32, tag="ln2_wb")
            nc.gpsimd.dma_start(ln1_wb[:], d_ln1[:])
            nc.gpsimd.dma_start(ln2_wb[:], d_ln2[:])
        if use_vb:
            bv_rep = small.tile([P, C], F32, tag="bv_rep")
            nc.gpsimd.dma_start(bv_rep[:], d_bvr[:])
        if use_mask:
            mask_sb = small.tile([P, NQ, T], F32, tag="mask")
            for tt in range(NQ):
                nc.gpsimd.dma_start(mask_sb[:, tt, :],
                                    d_mask[tt * P:(tt + 1) * P, :])

        def ln_tile(pool, src_ap, dst_bf_ap, wb, idx=0):
            """LayerNorm one [128, C] fp32 tile -> bf16 dst.

            The wide affine pass alternates between DVE and ACT by idx so
            neither engine serializes the tile pipeline.
            """
            stats = pool.tile([P, 2, 6], F32, tag="ln_stats")
            mv = pool.tile([P, 2], F32, tag="ln_mv")
            for g in range(2):
                nc.vector.bn_stats(stats[:, g, :],
                                   src_ap[:, g * 512:(g + 1) * 512])
            nc.vector.bn_aggr(mv[:], stats[:])
            std = pool.tile([P, 1], F32, tag="ln_std")
            nc.scalar.activation(std[:], mv[:, 1:2], AF.Sqrt, bias=eps_ap)
            rs = pool.tile([P, 1], F32, tag="ln_rs")
            nc.vector.reciprocal(rs[:], std[:])
            if wb is None:
                if idx % 2:
                    nc.vector.tensor_scalar(dst_bf_ap, src_ap, mv[:, 0:1],
                                            rs[:], op0=ALU.subtract,
                                            op1=ALU.mult)
                else:
                    # (x - mu) * rs == x*rs + (-mu*rs): wide pass on ACT
                    nbias = pool.tile([P, 1], F32, tag="ln_nb")
                    nc.vector.tensor_scalar(nbias[:], mv[:, 0:1], rs[:], -1.0,
                                            op0=ALU.mult, op1=ALU.mult)
                    nc.scalar.activation(dst_bf_ap, src_ap, AF.Identity,
                                         bias=nbias[:], scale=rs[:])
            else:
                tmp = pool.tile([P, C], F32, tag="ln_tmp")
                nc.vector.tensor_scalar(tmp[:], src_ap, mv[:, 0:1], rs[:],
                                        op0=ALU.subtract, op1=ALU.mult)
                nc.vector.scalar_tensor_tensor(tmp[:], tmp[:], 1.0,
                                               wb[:, 0, :],
                                               op0=ALU.mult, op1=ALU.mult)
                nc.vector.tensor_add(dst_bf_ap, tmp[:], wb[:, 1, :])

        def transpose_bf(src_ap, dst_tile, dst_ci_base, dst_col0, n, eng):
            """PE-transpose n [128,128] bf16 blocks of src_ap (free offset
            i*128) into dst_tile[:, dst_ci_base+i, dst_col0:+128].
            Groups of 4 share one PSUM bank and one copy instruction."""
            for g0 in range(0, n, 4):
                gn = min(4, n - g0)
                pst = ps_pool.tile([P, 4 * P], BF16, tag="ps")
                for i in range(gn):
                    nc.tensor.transpose(
                        pst[:, i * P:(i + 1) * P],
                        src_ap[:, (g0 + i) * P:(g0 + i + 1) * P], id_bf[:])
                dst = dst_tile[:, dst_ci_base + g0:dst_ci_base + g0 + gn,
                               dst_col0:dst_col0 + P]
                src = pst[:, 0:gn * P].rearrange("p (g f) -> p g f", g=gn)
                if eng == "v":
                    nc.vector.tensor_copy(dst, src)
                else:
                    nc.scalar.copy(dst, src)

        # ================= Phase A: LN1 + transposes ====================
        with tc.tile_pool(name="phA", bufs=5) as pa:
            # xq first so the q projection can start while the rest of
            # phase A is still running
            for tt in range(NQ):
                xt = pa.tile([P, C], F32, tag="x_t")
                nc.sync.dma_start(xt[:, 0:512],
                                  d_xq[tt * P:(tt + 1) * P, 0:512])
                nc.sync.dma_start(xt[:, 512:],
                                  d_xq[tt * P:(tt + 1) * P, 512:])
                hqb = pa.tile([P, C], BF16, tag="h_b")
                ln_tile(pa, xt[:], hqb[:], ln1_wb, tt)
                transpose_bf(hqb[:], hqT, 0, tt * P, 4, "v")
                transpose_bf(hqb[:, 4 * P:], hqT, 4, tt * P, 4, "s")
            for ti in range(NT):
                xt = pa.tile([P, C], F32, tag="x_t")
                nc.sync.dma_start(xt[:, 0:512],
                                  d_x[ti * P:(ti + 1) * P, 0:512])
                nc.sync.dma_start(xt[:, 512:],
                                  d_x[ti * P:(ti + 1) * P, 512:])
                hb = pa.tile([P, C], BF16, tag="h_b")
                ln_tile(pa, xt[:], hb[:], ln1_wb, ti)
                transpose_bf(hb[:], hT, 0, ti * P, 4, "v")
                transpose_bf(hb[:, 4 * P:], hT, 4, ti * P, 4, "s")

        # ================= Phase B: QKV projections =====================
        with tc.tile_pool(name="phB", bufs=5) as pb, \
             tc.tile_pool(name="phBv", bufs=1) as pbv:
            for m in range(NCH):
                wsb = pb.tile([P, C], BF16, tag="w_qk")
                nc.sync.dma_start(wsb[:], d_wq[m])
                ps = ps_pool.tile([P, TQ], F32, tag="ps")
                for kt in range(NCH):
                    nc.tensor.matmul(ps[:], wsb[:, kt * P:(kt + 1) * P],
                                     hqT[:, kt, :],
                                     start=(kt == 0), stop=(kt == NCH - 1))
                nc.scalar.activation(qT[:, m, :], ps[:], AF.Identity,
                                     bias=b_qkv[:, m:m + 1])
            for m in range(NCH):
                wsb = pb.tile([P, C], BF16, tag="w_qk")
                nc.sync.dma_start(wsb[:], d_wk[m])
                for nh in range(2):
                    ps = ps_pool.tile([P, 512], F32, tag="ps")
                    for kt in range(NCH):
                        nc.tensor.matmul(
                            ps[:], wsb[:, kt * P:(kt + 1) * P],
                            hT[:, kt, nh * 512:(nh + 1) * 512],
                            start=(kt == 0), stop=(kt == NCH - 1))
                    dst = kT[:, m, nh * 512:(nh + 1) * 512]
                    if (m + nh) % 2:
                        nc.scalar.activation(
                            dst, ps[:], AF.Identity,
                            bias=b_qkv[:, NCH + m:NCH + m + 1])
                    else:
                        nc.vector.tensor_scalar_add(
                            dst, ps[:], b_qkv[:, NCH + m:NCH + m + 1])
            wv_sb = pbv.tile([P, NCH, C], BF16, tag="w_v")
            for kt in range(NCH):
                nc.sync.dma_start(wv_sb[:, kt, :], d_wv[kt])
            for ti in range(NT):
                for nh in range(2):
                    ps = ps_pool.tile([P, 512], F32, tag="ps")
                    for kt in range(NCH):
                        nc.tensor.matmul(
                            ps[:], hT[:, kt, ti * P:(ti + 1) * P],
                            wv_sb[:, kt, nh * 512:(nh + 1) * 512],
                            start=(kt == 0), stop=(kt == NCH - 1))
                    dst = vtok[:, ti, nh * 512:(nh + 1) * 512]
                    if use_vb:
                        nc.vector.tensor_add(
                            dst, ps[:], bv_rep[:, nh * 512:(nh + 1) * 512])
                    elif (ti + nh) % 2:
                        nc.scalar.copy(dst, ps[:])
                    else:
                        nc.vector.tensor_copy(dst, ps[:])

        # ================= Phase C: attention ===========================
        # w_o weights prefetched here so phase D's matmuls are not stuck
        # behind the attention-output DMA backlog.
        es_wo = ExitStack()
        pdw = es_wo.enter_context(tc.tile_pool(name="phDw", bufs=NCH))
        wo_sb = []
        for m in range(NCH):
            w = pdw.tile([P, C], BF16, tag="w_o", name=f"w_o_{m}")
            nc.sync.dma_start(w[:], d_wo[m])
            wo_sb.append(w)

        with tc.tile_pool(name="phC", bufs=3) as pc_, \
             tc.tile_pool(name="phCt", bufs=2) as pct:
            for h in range(H):
                fi, po = h // 2, (h % 2) * HD
                attT = pct.tile([P, NCH, TQ], BF16, tag="attT")
                for qt in range(NQ):
                    ssc = sc_pool.tile([P, T], F32, tag="sc")
                    for nh in range(2):
                        nc.tensor.matmul(
                            ssc[:, nh * 512:(nh + 1) * 512],
                            qT[po:po + HD, fi, qt * P:(qt + 1) * P],
                            kT[po:po + HD, fi, nh * 512:(nh + 1) * 512],
                            start=True, stop=True)
                    ex = pc_.tile([P, T], F32, tag="ex")
                    den = pc_.tile([P, 1], F32, tag="den")
                    if use_mask:
                        exin = pc_.tile([P, T], F32, tag="exin")
                        nc.vector.tensor_add(exin[:], ssc[:],
                                             mask_sb[:, qt, :])
                        nc.scalar.activation(ex[:], exin[:], AF.Exp,
                                             scale=SM_SCALE,
                                             accum_out=den[:])
                    else:
                        nc.scalar.activation(ex[:], ssc[:], AF.Exp,
                                             scale=SM_SCALE,
                                             accum_out=den[:])
                    rec = pc_.tile([P, 1], F32, tag="rec")
                    nc.vector.reciprocal(rec[:], den[:])
                    att = pc_.tile([P, T], F32, tag="att")
                    nc.vector.tensor_scalar_mul(att[:], ex[:], rec[:])
                    nc.gpsimd.dma_start(d_att[h, qt * P:(qt + 1) * P, :],
                                        att[:])
                    for g0 in range(0, NCH, 4):
                        pst = ps_pool.tile([P, 512], F32, tag="ps")
                        for i in range(4):
                            nc.tensor.transpose(
                                pst[:, i * P:(i + 1) * P],
                                att[:, (g0 + i) * P:(g0 + i + 1) * P],
                                id_f[:])
                        dst = attT[:, g0:g0 + 4, qt * P:(qt + 1) * P]
                        src = pst[:].rearrange("p (g f) -> p g f", g=4)
                        if (qt + g0 // 4) % 2:
                            nc.scalar.copy(dst, src)
                        else:
                            nc.vector.tensor_copy(dst, src)
                ps_y = ps_pool.tile([HD, TQ], F32, tag="ps")
                v0 = fi * P + po
                for kt in range(NCH):
                    nc.tensor.matmul(ps_y[:], vtok[:, kt, v0:v0 + HD],
                                     attT[:, kt, :],
                                     start=(kt == 0), stop=(kt == NCH - 1))
                nc.vector.tensor_copy(yT[po:po + HD, fi, :], ps_y[:])

        # ============ Phase D: output proj + residual + LN2 =============
        with tc.tile_pool(name="phD", bufs=3) as pd_, \
             tc.tile_pool(name="phDao", bufs=1) as pdao:
            aoT = pdao.tile([P, NCH, TQ], BF16, tag="aoT")
            for m in range(NCH):
                wsb = wo_sb[m]
                ps = ps_pool.tile([P, TQ], F32, tag="ps")
                for kt in range(NCH):
                    nc.tensor.matmul(ps[:], wsb[:, kt * P:(kt + 1) * P],
                                     yT[:, kt, :],
                                     start=(kt == 0), stop=(kt == NCH - 1))
                nc.scalar.activation(aoT[:, m, :], ps[:], AF.Identity,
                                     bias=b_o[:, m:m + 1])
            # 1) all residual transposes+adds, 2) all LN2s, 3) all h2T
            # transposes — keeps consecutive tt chains off the PE stream's
            # critical path.
            for tt in range(NQ):
                xqt = pd_.tile([P, C], F32, tag="xq_t", bufs=4)
                nc.sync.dma_start(xqt[:, 0:512], d_xq[tt * P:(tt + 1) * P,
                                                      0:512])
                nc.sync.dma_start(xqt[:, 512:], d_xq[tt * P:(tt + 1) * P,
                                                     512:])
                pst = ps_pool.tile([P, C], BF16, tag="ps")
                for m in range(NCH):
                    nc.tensor.transpose(pst[:, m * P:(m + 1) * P],
                                        aoT[:, m, tt * P:(tt + 1) * P],
                                        id_bf[:])
                nc.vector.tensor_add(xmids[tt][:], pst[:], xqt[:])
            h2bs = []
            for tt in range(NQ):
                h2b = pd_.tile([P, C], BF16, tag="h2_b", bufs=4,
                               name=f"h2b{tt}")
                ln_tile(pd_, xmids[tt][:], h2b[:], ln2_wb, tt)
                h2bs.append(h2b)
            for tt in range(NQ):
                transpose_bf(h2bs[tt][:], h2T, 0, tt * P, 4, "v")
                transpose_bf(h2bs[tt][:, 4 * P:], h2T, 4, tt * P, 4, "s")
        es_wo.close()

        # ================= Phase E: FFN =================================
        with tc.tile_pool(name="phE", bufs=3) as pe, \
             tc.tile_pool(name="phEw", bufs=6) as pew:
            for m in range(NF):
                wsb = pew.tile([P, C], BF16, tag="w_fc")
                nc.sync.dma_start(wsb[:], d_wfc[m])
                ps = ps_pool.tile([P, TQ], F32, tag="ps")
                for kt in range(NCH):
                    nc.tensor.matmul(ps[:], wsb[:, kt * P:(kt + 1) * P],
                                     h2T[:, kt, :],
                                     start=(kt == 0), stop=(kt == NCH - 1))
                nc.scalar.activation(gT[:, m, :], ps[:], AF.Gelu,
                                     bias=b_fc[:, m:m + 1])
            for m in range(NCH):
                wsb = pe.tile([P, 4 * C], BF16, tag="w_pj")
                nc.sync.dma_start(wsb[:], d_wpj[m])
                ps = ps_pool.tile([P, TQ], F32, tag="ps")
                for kt in range(NF):
                    nc.tensor.matmul(ps[:], wsb[:, kt * P:(kt + 1) * P],
                                     gT[:, kt, :],
                                     start=(kt == 0), stop=(kt == NF - 1))
                nc.scalar.activation(pT[:, m, :], ps[:], AF.Identity,
                                     bias=b_pj[:, m:m + 1])
            for tt in range(NQ):
                pst = ps_pool.tile([P, C], BF16, tag="ps")
                for m in range(NCH):
                    nc.tensor.transpose(pst[:, m * P:(m + 1) * P],
                                        pT[:, m, tt * P:(tt + 1) * P],
                                        id_bf[:])
                xo = pe.tile([P, C], F32, tag="xo")
                nc.vector.tensor_add(xo[:], pst[:], xmids[tt][:])
                nc.sync.dma_start(d_xo[tt * P:(tt + 1) * P, :], xo[:])

    nc.compile()
    return nc


_NC_CACHE = {}
_LAST_IN_MAPS = None


def _get_nc(key):
    if key not in _NC_CACHE:
        _NC_CACHE[key] = build_nc(*key)
    return _NC_CACHE[key]


def kernel(x, attention_mask, ln1_w, ln1_b, w_qkv, b_qkv, w_o, b_o,
           ln2_w, ln2_b, w_fc, b_fc, w_proj, b_proj):
    x = np.asarray(x, np.float32)
    attention_mask = np.asarray(attention_mask, np.float32)
    ln1_w = np.asarray(ln1_w, np.float32)
    ln1_b = np.asarray(ln1_b, np.float32)
    w_qkv = np.asarray(w_qkv, np.float32)
    b_qkv_a = np.asarray(b_qkv, np.float32)
    w_o = np.asarray(w_o, np.float32)
    b_o_a = np.asarray(b_o, np.float32)
    ln2_w = np.asarray(ln2_w, np.float32)
    ln2_b = np.asarray(ln2_b, np.float32)
    w_fc = np.asarray(w_fc, np.float32)
    b_fc_a = np.asarray(b_fc, np.float32)
    w_proj = np.asarray(w_proj, np.float32)
    b_proj_a = np.asarray(b_proj, np.float32)

    use_mask = bool(np.any(attention_mask))
    use_ln_affine = not (np.all(ln1_w == 1) and np.all(ln1_b == 0)
                         and np.all(ln2_w == 1) and np.all(ln2_b == 0))
    use_vb = bool(np.any(b_qkv_a[2 * C:]))

    nc = _get_nc((use_mask, use_ln_affine, use_vb))

    shared = {
        "wq4": _w4(w_qkv[0:C]),
        "wk4": _w4(w_qkv[C:2 * C]),
        "wvr": np.ascontiguousarray(
            w_qkv[2 * C:3 * C].T.reshape(NCH, P, C)).astype(_BF),
        "wo4": _w4(w_o),
        "wfc4": _w4(w_fc),
        "wpj4": _w4(w_proj),
        "bqkv": np.ascontiguousarray(b_qkv_a.reshape(3 * NCH, P).T),
        "bo": np.ascontiguousarray(b_o_a.reshape(NCH, P).T),
        "bfc": np.ascontiguousarray(b_fc_a.reshape(NF, P).T),
        "bpj": np.ascontiguousarray(b_proj_a.reshape(NCH, P).T),
        "ident_bf": np.eye(P, dtype=_BF),
        "ident_f": np.eye(P, dtype=np.float32),
    }
    if use_ln_affine:
        shared["ln1_wb"] = np.ascontiguousarray(np.broadcast_to(
            np.stack([ln1_w, ln1_b]), (P, 2, C)))
        shared["ln2_wb"] = np.ascontiguousarray(np.broadcast_to(
            np.stack([ln2_w, ln2_b]), (P, 2, C)))
    if use_vb:
        shared["bv_rep"] = np.ascontiguousarray(
            np.broadcast_to(b_qkv_a[2 * C:], (P, C)))

    in_maps = []
    for core in range(8):
        b, qh = core // 2, core % 2
        m = dict(shared)
        m["x_full"] = np.ascontiguousarray(x[b])
        m["xq"] = np.ascontiguousarray(x[b, qh * TQ:(qh + 1) * TQ])
        if use_mask:
            # activation computes exp(scale*(s + m')), so pre-divide the
            # mask by scale to get exp(scale*s + mask).
            m["mask_q"] = np.ascontiguousarray(
                np.broadcast_to(attention_mask[0, 0], (T, T))
                [qh * TQ:(qh + 1) * TQ] / SM_SCALE)
        in_maps.append(m)

    global _LAST_IN_MAPS
    _LAST_IN_MAPS = in_maps
    res = run_bass_kernel_spmd(nc, in_maps, list(range(8)))

    x_out = np.empty((B, T, C), np.float32)
    att = np.empty((B, H, T, T), np.float32)
    for core in range(8):
        b, qh = core // 2, core % 2
        x_out[b, qh * TQ:(qh + 1) * TQ] = res.results[core]["x_out"]
        att[b, :, qh * TQ:(qh + 1) * TQ, :] = res.results[core]["att_out"]
    return (x_out, att)
